# revision 1
# baseline (speedup 1.0000x reference)
"""Trainium2 Bass kernel for nn_ProtoCycleModel (retrieval_knn).

Problem: P=65536 prototypes, C=64 classes, D=256.
Per class c (rows c::64 of each table, n=1024):
    p2_inv = (p2_c - b) @ inv(W.T)          # y-side of direction "source"
    p1_fwd = p1_c @ W.T + b                 # y-side of direction "target"
    loss_src[c] = mean_i min_j ||p1_c[i] - p2_inv[j]||^2
    loss_tgt[c] = mean_i min_j ||p2_c[i] - p1_fwd[j]||^2
Output: (2, 64) fp32.

Sharding: class axis across 8 cores (8 classes/core). Each core:
  - loads its (8*1024, 256) slices of both tables (row-major, contiguous)
  - PE-transposes them to d-major (fp32 exact)
  - computes transformed tables directly in transposed space:
        yT = Mat @ xT + bias   (Mat = -2*inv(W.T)-style, folded scale -2)
    so the pairwise matmul G = xT.T @ yT gives -2 * x.y' directly.
  - |y'|^2 row: ones-matmul over Square(transform psum + bias) (scale 1/4
    baked into a 0.25-constant stationary matrix), broadcast to all 128
    partitions for free via M=128 stationary ones.
  - per i-tile: fused DVE tensor_tensor_reduce: min_j (G + |y'|^2) -> [128,1]
  - per-class scalars via ones-matmul cross-partition sum; host gathers.

All matmuls run in float32r (TF32-like, full PE rate at N>=512, ~16x more
accurate than bf16). Everything else fp32.
"""

import numpy as np

P, C, D = 65536, 64, 256
N_CORES = 8
CPC = C // N_CORES          # classes per core = 8
NPC = P // C                # prototypes per class = 1024
IT = NPC // 128             # i-tiles per class = 8

# ys application mode: "ttr" = fused DVE tensor_tensor_reduce;
# "fold" = K=1 matmul folds ys row into PSUM, then plain tensor_reduce.
YS_MODE = "fold"
import os as _os
PSG_WIDE = _os.environ.get("K_PSG_WIDE", "0") == "1"   # [128,1024] G tiles
PSG_BUFS = int(_os.environ.get("K_PSG_BUFS", "4"))
PSM_BUFS = int(_os.environ.get("K_PSM_BUFS", "2"))

_CACHE = {}


def _build_bass():
    import concourse.bass as bass
    from concourse import bacc
    import concourse.tile as tile
    from concourse import mybir
    from concourse.masks import make_identity

    FP32 = mybir.dt.float32
    FP32R = mybir.dt.float32r
    BF16 = mybir.dt.bfloat16
    AF = mybir.ActivationFunctionType
    ALU = mybir.AluOpType
    AX = mybir.AxisListType

    nc = bacc.Bacc(None, target_bir_lowering=False)

    p1_d = nc.dram_tensor("p1", [CPC * NPC, D], FP32, kind="ExternalInput")
    p2_d = nc.dram_tensor("p2", [CPC * NPC, D], FP32, kind="ExternalInput")
    # mats[dir][kchunk] : [128, 256] fp32, lhsT layout [d, d'] with the -2
    # scale folded in.  dir 0 = source (V2 = -2*inv(W.T)), dir 1 = target
    # (Wt2 = -2*W.T).
    mats_d = nc.dram_tensor("mats", [2, 2, 128, D], FP32, kind="ExternalInput")
    consts_d = nc.dram_tensor("consts", [128, 385], FP32, kind="ExternalInput")
    # biases[dir] : [128, 2] fp32 (column = d' chunk);  dir0 = +2*(b@V),
    # dir1 = -2*b.
    bias_d = nc.dram_tensor("biases", [2, 128, 2], FP32, kind="ExternalInput")
    out_d = nc.dram_tensor("out", [1, 2 * CPC], FP32, kind="ExternalOutput")

    with tile.TileContext(nc) as tc:
        with (
            tc.tile_pool(name="const", bufs=1) as const,
            tc.tile_pool(name="xrow", bufs=6) as xrow_p,
            tc.tile_pool(name="xt", bufs=10) as xt_p,
            tc.tile_pool(name="yt", bufs=8) as yt_p,
            tc.tile_pool(name="sq", bufs=4) as sq_p,
            tc.tile_pool(name="ysb", bufs=4) as ysb_p,
            tc.tile_pool(name="scr", bufs=3) as scr_p,
            tc.tile_pool(name="psg", bufs=PSG_BUFS, space="PSUM") as psg_p,
            tc.tile_pool(name="psm", bufs=PSM_BUFS, space="PSUM") as psm_p,
        ):
            # ---- constants ----
            cblk_raw = const.tile([128, 385], FP32)
            nc.scalar.dma_start(cblk_raw[:], consts_d[:])
            cblk = const.tile([128, 385], FP32R)
            nc.vector.tensor_copy(cblk[:], cblk_raw[:])

            mats_raw = const.tile([128, 2, 2, D], FP32)
            nc.scalar.dma_start(mats_raw[:], mats_d[:].rearrange("a b p d -> p a b d"))
            mats = const.tile([128, 2, 2, D], FP32R)
            nc.vector.tensor_copy(mats[:], mats_raw[:])

            biases = const.tile([128, 2, 2], FP32)  # [p, dir, dchunk]
            nc.scalar.dma_start(biases[:], bias_d[:].rearrange("a p c -> p a c"))
            identr = cblk[:, 0:128]
            identf = cblk_raw[:, 0:128]
            ones1r = cblk[:, 128:129]
            ones_q = cblk[:, 257:385]

            pmin = const.tile([128, 2 * CPC * IT], FP32)   # col = dir*64+c*8+it
            pmin2 = (const.tile([128, 2 * CPC * IT], FP32, name="pmin2")
                     if not PSG_WIDE else pmin)
            pxs = const.tile([128, 2 * CPC * 2], FP32)     # col = dir*16+c*2+dc

            onesrow = cblk[0:1, 128:256]

            # ---- main loop: software-pipelined (prep one class ahead) ----
            state = {}

            def prep(c):
                xts = [[None, None], [None, None]]  # [table][dchunk]
                for t in range(2):
                    src_d = p1_d if t == 0 else p2_d
                    xr = xrow_p.tile([128, IT, D], FP32, tag="xrow", bufs=3)
                    xrr = xrow_p.tile([128, IT, D], FP32R, tag="xrowr", bufs=4)
                    half = NPC // 2
                    for hh in range(2):
                        nc.sync.dma_start(
                            xr[:, hh * (IT // 2):(hh + 1) * (IT // 2), :],
                            src_d[c * NPC + hh * half:
                                  c * NPC + (hh + 1) * half, :].rearrange(
                                "(k p) d -> p k d", p=128),
                        )
                        nc.vector.tensor_copy(
                            xrr[:, hh * (IT // 2):(hh + 1) * (IT // 2), :],
                            xr[:, hh * (IT // 2):(hh + 1) * (IT // 2), :])
                    for dc in range(2):
                        pst = psm_p.tile([128, 1024], FP32R, tag="misc")
                        for k in range(IT):
                            nc.tensor.transpose(
                                pst[:, k * 128:(k + 1) * 128],
                                xrr[:, k, dc * 128:(dc + 1) * 128],
                                identr,
                            )
                        xt_t = xt_p.tile([128, NPC], FP32R, tag="xt")
                        nc.scalar.copy(xt_t[:], pst[:])
                        xts[t][dc] = xt_t
                        # xs partials: sum_i x^2 per d-partition
                        trash = scr_p.tile([128, NPC], BF16, tag="scr")
                        nc.scalar.activation(
                            trash[:], xt_t[:], AF.Square,
                            accum_out=pxs[:, t * 16 + c * 2 + dc:
                                          t * 16 + c * 2 + dc + 1],
                        )

                yts_all = [[], []]
                ysrow_all = [None, None]
                for dr in range(2):
                    ysrc = xts[1 - dr]    # dir0: y from p2; dir1: y from p1
                    sqs = []
                    for dcp in range(2):   # output d' chunk
                        pstf = psm_p.tile([128, 1024], FP32, tag="misc")
                        for dc in range(2):
                            for ih in range(2):
                                nc.tensor.matmul(
                                    pstf[:, ih * 512:(ih + 1) * 512],
                                    mats[:, dr, dc, dcp * 128:(dcp + 1) * 128],
                                    ysrc[dc][:, ih * 512:(ih + 1) * 512],
                                    start=(dc == 0), stop=(dc == 1),
                                )
                        bias_ap = biases[:, dr, dcp:dcp + 1]
                        yt_t = yt_p.tile([128, NPC], FP32R, tag="yt")
                        nc.scalar.activation(
                            yt_t[:], pstf[:], AF.Identity, bias=bias_ap, scale=1.0)
                        sq_t = sq_p.tile([128, NPC], FP32R, tag="sq")
                        nc.scalar.activation(
                            sq_t[:], pstf[:], AF.Square, bias=bias_ap, scale=1.0)
                        yts_all[dr].append(yt_t)
                        sqs.append(sq_t)

                    psy = psm_p.tile([128, 1024], FP32, tag="misc")
                    for jh in range(2):
                        for dcp in range(2):
                            nc.tensor.matmul(
                                psy[0:1, jh * 512:(jh + 1) * 512],
                                ones_q[:, 0:1],
                                sqs[dcp][:, jh * 512:(jh + 1) * 512],
                                start=(dcp == 0), stop=(dcp == 1),
                            )
                    ysrow = ysb_p.tile([1, NPC], FP32R, tag="ysrow")
                    nc.scalar.copy(ysrow[:], psy[0:1, :])
                    ysrow_all[dr] = ysrow
                state[c] = (xts, yts_all, ysrow_all)

            def pairwise(c):
                xts, yts_all, ysrow_all = state.pop(c)
                for dr in range(2):
                    xside = xts[dr]       # dir0: x = p1; dir1: x = p2
                    yts = yts_all[dr]
                    ysrow = ysrow_all[dr]
                    for it in range(IT):
                        col = dr * 64 + c * 8 + it
                        pgs = [psg_p.tile([128, 512], FP32, tag="g",
                                          name=f"pg{jh}")
                               for jh in range(2)]
                        for dc in range(2):          # stationary reused 2x
                            for jh in range(2):
                                nc.tensor.matmul(
                                    pgs[jh][:],
                                    xside[dc][:, it * 128:(it + 1) * 128],
                                    yts[dc][:, jh * 512:(jh + 1) * 512],
                                    start=(dc == 0), stop=False,
                                )
                        for jh in range(2):          # ys fold, ones stationary
                            nc.tensor.matmul(
                                pgs[jh][:],
                                onesrow,
                                ysrow[:, jh * 512:(jh + 1) * 512],
                                start=False, stop=True,
                            )
                        for jh in range(2):
                            dst = pmin if jh == 0 else pmin2
                            nc.vector.tensor_reduce(
                                out=dst[:, col:col + 1], in_=pgs[jh][:],
                                axis=AX.X, op=ALU.min,
                            )

            prep(0)
            for c in range(CPC):
                if c + 1 < CPC:
                    prep(c + 1)
                pairwise(c)

            # ---- finals ----
            if PSG_WIDE:
                pminc = pmin
            else:
                pminc = const.tile([128, 2 * CPC * IT], FP32, name="pminc")
                nc.vector.tensor_tensor(
                    out=pminc[:], in0=pmin[:], in1=pmin2[:], op=ALU.min)
            red_min = const.tile([128, 16], FP32)
            nc.vector.tensor_reduce(
                out=red_min[:], in_=pminc[:].rearrange("p (g k) -> p g k", k=IT),
                axis=AX.X, op=ALU.add)
            red_xs = const.tile([128, 16], FP32)
            nc.vector.tensor_reduce(
                out=red_xs[:], in_=pxs[:].rearrange("p (g k) -> p g k", k=2),
                axis=AX.X, op=ALU.add)
            red = const.tile([128, 16], FP32R)
            nc.vector.tensor_tensor(
                out=red[:], in0=red_min[:], in1=red_xs[:], op=ALU.add)
            psf = psm_p.tile([1, 16], FP32, tag="misc")
            nc.tensor.matmul(psf[:], ones1r, red[:], start=True, stop=True)
            outrow = const.tile([1, 16], FP32)
            nc.scalar.mul(outrow[:], psf[:], 1.0 / NPC)
            nc.sync.dma_start(out_d[:], outrow[:])

    nc.compile()
    return nc


def _get_nc():
    if "nc" not in _CACHE:
        _CACHE["nc"] = _build_bass()
    return _CACHE["nc"]


def kernel(protos1, protos2, W, b, num_classes):
    from concourse.bass_utils import run_bass_kernel_spmd

    nc_classes = int(num_classes)
    assert nc_classes == C and protos1.shape == (P, D)

    protos1 = np.ascontiguousarray(protos1, dtype=np.float32)
    protos2 = np.ascontiguousarray(protos2, dtype=np.float32)
    W = np.asarray(W, dtype=np.float32)
    b = np.asarray(b, dtype=np.float32)

    # host-side tiny prep: inverse + scaled transform matrices
    V = np.linalg.inv(W.T.astype(np.float64)).astype(np.float32)  # (p2-b)@V
    V2 = (-2.0 * V).astype(np.float32)                 # lhsT [d, d'] dir0
    Wt2 = (-2.0 * W.T).astype(np.float32)              # lhsT [d, d'] dir1
    bias0 = (2.0 * (b.astype(np.float64) @ V.astype(np.float64))).astype(np.float32)
    bias1 = (-2.0 * b).astype(np.float32)
    mats = np.stack([
        np.stack([V2[0:128, :], V2[128:256, :]]),
        np.stack([Wt2[0:128, :], Wt2[128:256, :]]),
    ]).astype(np.float32)                               # [2, 2, 128, 256]
    idb = np.eye(128, dtype=np.float32)
    consts = np.concatenate([
        idb,
        np.ones((128, 129), dtype=np.float32),
        np.full((128, 128), 0.25, dtype=np.float32),
    ], axis=1)
    biases = np.stack([
        bias0.reshape(2, 128).T,                        # [128, 2] cols = chunk
        bias1.reshape(2, 128).T,
    ]).astype(np.float32)                               # [2, 128, 2]

    # class-major reordering: (P, D) -> (C, NPC, D)
    p1c = np.ascontiguousarray(protos1.reshape(NPC, C, D).transpose(1, 0, 2))
    p2c = np.ascontiguousarray(protos2.reshape(NPC, C, D).transpose(1, 0, 2))

    in_maps = []
    for core in range(N_CORES):
        sl = slice(core * CPC, (core + 1) * CPC)
        in_maps.append({
            "p1": np.ascontiguousarray(p1c[sl].reshape(CPC * NPC, D)),
            "p2": np.ascontiguousarray(p2c[sl].reshape(CPC * NPC, D)),
            "mats": mats,
            "biases": biases,
            "consts": consts,
        })

    nc = _get_nc()
    res = run_bass_kernel_spmd(nc, in_maps, core_ids=list(range(N_CORES)))
    _CACHE["last_result"] = res

    out = np.zeros((2, C), dtype=np.float32)
    for core in range(N_CORES):
        row = res.results[core]["out"].reshape(2, CPC)
        out[:, core * CPC:(core + 1) * CPC] = row
    return out



# revision 14
# speedup vs baseline: 1.1620x; 1.1620x over previous
"""Trainium2 Bass kernel for nn_ProtoCycleModel (retrieval_knn), v2.

Problem: P=65536 prototypes, C=64 classes, D=256.
Per class c (rows c::64 of each table, n=1024):
    loss_src[c] = mean_i min_j ||p1_c[i] - inv(W.T)@(p2_c[j]-b)||^2
    loss_tgt[c] = mean_i min_j ||p2_c[i] - (W.T@p1_c[j]+b)||^2
Output: (2, 64) fp32.  Sharding: 8 classes per core.

Design ("flipped layout"):
  - Host sends tables d-major (C, 2, 128, NPC) as fp32(r) AND fp8e4
    (scaled by power-of-2 sx); host also precomputes mean|x|^2 per class
    (added to the device result at the end, like the inv(W) prep).
  - Transform y' = M@x + b on PE in fp32r -> yt8 (fp8, scale sy_dr) via ACT.
  - sq = yt8^2 (ACT or Pool), ysrow = ones^T sq (PE, value sx/(4 sy)) ->
    ys scatter-DMA'd from the [1,1024] psum row into [128, 8] columns.
  - Pairwise G'[j%128, i] = sum_d qx[d,i] * yt8[d,j]: ONE fp8 DoubleRow
    matmul per 128-j tile (K=256 in one pass, 0.5 cycles/row).
  - j sits on PSUM partitions, so +|y'|^2 is a per-partition scalar:
    DVE scalar_tensor_tensor fuses (G + ys) and running min across
    j-tiles in the single required PSUM pass; ACT-assigned class-dirs
    use activation(bias=ys_col) copies merged by Pool tensor_tensor min.
  - Finish per class-dir: 8 PE transposes of the [128,1024] bf16 running
    min -> psum [128, 8, 128], one DVE min-reduce -> pmin columns;
    final: add-reduce, ones-matmul cross-partition sum, scale, DMA out.
"""

import math
import os

import numpy as np

P, C, D = 65536, 64, 256
N_CORES = 8
CPC = C // N_CORES          # classes per core = 8
NPC = P // C                # prototypes per class = 1024
JT = NPC // 128             # j-tiles per class-dir = 8

# dir1 classes whose stream runs on ACT+Pool instead of DVE
N_ACT_CDS = int(os.environ.get("K_ACT_CDS", "10"))
# sq on Pool for dir0 (else ACT)
SQ_POOL_D0 = os.environ.get("K_SQ_POOL_D0", "0") == "1"

_CACHE = {}


def _build_bass():
    from concourse import bacc
    import concourse.tile as tile
    from concourse import mybir

    FP32 = mybir.dt.float32
    FP32R = mybir.dt.float32r
    BF16 = mybir.dt.bfloat16
    FP8 = mybir.dt.float8e4
    AF = mybir.ActivationFunctionType
    ALU = mybir.AluOpType
    AX = mybir.AxisListType
    PM = mybir.MatmulPerfMode

    nc = bacc.Bacc(None, target_bir_lowering=False)

    p1t_d = nc.dram_tensor("p1t", [CPC, 2, 128, NPC], FP32R, kind="ExternalInput")
    p2t_d = nc.dram_tensor("p2t", [CPC, 2, 128, NPC], FP32R, kind="ExternalInput")
    q1t_d = nc.dram_tensor("q1t", [CPC, 2, 128, NPC], FP8, kind="ExternalInput")
    q2t_d = nc.dram_tensor("q2t", [CPC, 2, 128, NPC], FP8, kind="ExternalInput")
    # mats[dir][dc]: [128, 256] fp32r, lhsT [d, d'] with -2 folded in
    mats_d = nc.dram_tensor("mats", [2, 2, 128, D], FP32R, kind="ExternalInput")
    # biases[dir][dcp] per-partition: sy_dr * bias_raw_dr
    bias_d = nc.dram_tensor("biases", [2, 128, 3], FP32, kind="ExternalInput")
    # consts cols: 0 = ones 1.0, 1..2 = sx/(4*sy_dr)
    ones_d = nc.dram_tensor("onesc", [128, 3], FP32R, kind="ExternalInput")
    idb_d = nc.dram_tensor("idb", [128, 128], BF16, kind="ExternalInput")
    finsc_d = nc.dram_tensor("finsc", [1, 2 * CPC], FP32, kind="ExternalInput")
    onef_d = nc.dram_tensor("onef", [1, 2], FP32, kind="ExternalInput")
    out_d = nc.dram_tensor("out", [1, 2 * CPC], FP32, kind="ExternalOutput")
    DEBUG = os.environ.get("K_DEBUG", "0") == "1"
    if DEBUG:
        dpmin_d = nc.dram_tensor("dpmin", [128, 2 * CPC * JT], FP32,
                                 kind="ExternalOutput")
        dysc_d = nc.dram_tensor("dysc", [2, 128, JT], FP32,
                                kind="ExternalOutput")
        dyt8_d = nc.dram_tensor("dyt8", [128, 2, NPC], FP32,
                                kind="ExternalOutput")

    act_cds = {(c, 1) for c in range(min(N_ACT_CDS, 8))} | {
        (c, 0) for c in range(max(0, N_ACT_CDS - 8))}

    with tile.TileContext(nc) as tc:
        with (
            tc.tile_pool(name="const", bufs=1) as const,
            tc.tile_pool(name="xt", bufs=2) as xt_p,
            tc.tile_pool(name="qx", bufs=2) as qx_p,
            tc.tile_pool(name="yt", bufs=2) as yt_p,
            tc.tile_pool(name="sq", bufs=2) as sq_p,
            tc.tile_pool(name="ysc", bufs=2) as ysc_p,
            tc.tile_pool(name="run", bufs=3) as run_p,
            tc.tile_pool(name="gb", bufs=4) as gb_p,
            tc.tile_pool(name="mg", bufs=3) as mg_p,
            tc.tile_pool(name="psg", bufs=2, space="PSUM") as psg_p,
            tc.tile_pool(name="psx", bufs=2, space="PSUM") as psx_p,
        ):
            # ---- constants ----
            mats = const.tile([128, 2, 2, D], FP32R)
            nc.sync.dma_start(mats[:], mats_d[:].rearrange("a b p d -> p a b d"))
            biases = const.tile([128, 2, 3], FP32)
            nc.sync.dma_start(biases[:], bias_d[:].rearrange("a p c -> p a c"))
            onesc = const.tile([128, 3], FP32R)
            nc.sync.dma_start(onesc[:], ones_d[:])
            idb = const.tile([128, 128], BF16)
            nc.sync.dma_start(idb[:], idb_d[:])
            finsc = const.tile([1, 2 * CPC], FP32)
            nc.sync.dma_start(finsc[:], finsc_d[:])
            onef = const.tile([1, 2], FP32)
            nc.sync.dma_start(onef[:], onef_d[:])

            pmin = const.tile([128, 2 * CPC * JT], FP32)  # col = dr*64+c*8+ib

            state = {}

            def prep(c):
                xts, qxs = [], []
                for t, (src_d, qsrc_d) in enumerate(
                    ((p1t_d, q1t_d), (p2t_d, q2t_d))
                ):
                    xt = xt_p.tile([128, 2, NPC], FP32R, tag=f"xt{t}")
                    nc.sync.dma_start(
                        xt[:], src_d[c].rearrange("a p j -> p a j"))
                    qx = qx_p.tile([128, 2, NPC], FP8, tag=f"qx{t}")
                    nc.sync.dma_start(
                        qx[:], qsrc_d[c].rearrange("a p j -> p a j"))
                    xts.append(xt)
                    qxs.append(qx)

                yt8s, yscs = [], []
                for dr in range(2):
                    ysrc = xts[1 - dr]   # dir0: y from p2; dir1: y from p1
                    yt8 = yt_p.tile([128, 2, NPC], FP8, tag=f"yt{dr}")
                    for dcp in range(2):
                        pstf = psx_p.tile([128, NPC], FP32, tag="xf")
                        for dc in range(2):
                            for ih in range(2):
                                nc.tensor.matmul(
                                    pstf[:, ih * 512:(ih + 1) * 512],
                                    mats[:, dr, dc, dcp * 128:(dcp + 1) * 128],
                                    ysrc[:, dc, ih * 512:(ih + 1) * 512],
                                    start=(dc == 0), stop=(dc == 1),
                                )
                        nc.scalar.activation(
                            yt8[:, dcp, :], pstf[:], AF.Identity,
                            bias=biases[:, dr, dcp:dcp + 1],
                            scale=biases[:, dr, 2:3])
                    sq = sq_p.tile([128, 2, NPC], FP32R, tag=f"sq{dr}")
                    if SQ_POOL_D0:
                        nc.gpsimd.tensor_tensor(
                            out=sq[:], in0=yt8[:], in1=yt8[:], op=ALU.mult)
                    else:
                        nc.scalar.activation(sq[:], yt8[:], AF.Square)
                    ysp = psx_p.tile([1, NPC], FP32, tag="xf")
                    for jh in range(2):
                        for dcp in range(2):
                            nc.tensor.matmul(
                                ysp[:, jh * 512:(jh + 1) * 512],
                                onesc[:, 1 + dr:2 + dr],
                                sq[:, dcp, jh * 512:(jh + 1) * 512],
                                start=(dcp == 0), stop=(dcp == 1),
                            )
                    ysr = ysc_p.tile([1, NPC], FP32, tag=f"ysr{dr}")
                    nc.scalar.copy(ysr[:], ysp[:])
                    ysp2 = psx_p.tile([128, JT], FP32, tag="xf")
                    for jt in range(JT):
                        nc.tensor.matmul(
                            ysp2[:, jt:jt + 1],
                            ysr[:, jt * 128:(jt + 1) * 128],
                            onef[0:1, 0:1],
                            start=True, stop=True,
                        )
                    ysc = ysc_p.tile([128, JT], FP32, tag=f"ys{dr}")
                    nc.vector.tensor_copy(ysc[:], ysp2[:])
                    if DEBUG and c == 0:
                        nc.sync.dma_start(dysc_d[dr], ysc[:])
                        if dr == 0:
                            dy = const.tile([128, 2, NPC], FP32, name="dy")
                            nc.vector.tensor_copy(dy[:], yt8[:])
                            nc.sync.dma_start(dyt8_d[:], dy[:])
                    yt8s.append(yt8)
                    yscs.append(ysc)
                state[c] = (qxs, yt8s, yscs)

            def pairwise(c):
                qxs, yt8s, yscs = state.pop(c)
                runs = [None, None]
                for jt in range(JT):
                    for dr in range(2):
                        g = psg_p.tile([128, NPC], FP32, tag="g")
                        for jh in range(2):
                            nc.tensor.matmul(
                                g[:, jh * 512:(jh + 1) * 512],
                                yt8s[dr][:, :, jt * 128:(jt + 1) * 128],
                                qxs[dr][:, :, jh * 512:(jh + 1) * 512],
                                start=True, stop=True,
                                perf_mode=PM.DoubleRow,
                            )
                        ys_col = yscs[dr][:, jt:jt + 1]
                        if (c, dr) in act_cds:
                            gb = gb_p.tile([128, NPC], BF16, tag="gb")
                            nc.scalar.activation(
                                gb[:], g[:], AF.Identity, bias=ys_col,
                                scale=1.0)
                            if jt == 0:
                                runs[dr] = gb
                            else:
                                mg = mg_p.tile([128, NPC], BF16, tag="mg")
                                nc.vector.tensor_tensor(
                                    out=mg[:], in0=runs[dr][:], in1=gb[:],
                                    op=ALU.min)
                                runs[dr] = mg
                        else:
                            nrun = run_p.tile([128, NPC], BF16, tag="run")
                            if jt == 0:
                                nc.vector.tensor_scalar(
                                    out=nrun[:], in0=g[:], scalar1=ys_col,
                                    scalar2=None, op0=ALU.add)
                            else:
                                nc.vector.scalar_tensor_tensor(
                                    out=nrun[:], in0=g[:], scalar=ys_col,
                                    in1=runs[dr][:], op0=ALU.add,
                                    op1=ALU.min)
                            runs[dr] = nrun
                for dr in range(2):
                    ft = psg_p.tile([128, JT, 128], BF16, tag="g")
                    for ib in range(JT):
                        nc.tensor.transpose(
                            ft[:, ib, :],
                            runs[dr][:, ib * 128:(ib + 1) * 128],
                            idb[:],
                        )
                    base = dr * 64 + c * 8
                    nc.vector.tensor_reduce(
                        out=pmin[:, base:base + JT], in_=ft[:],
                        axis=AX.X, op=ALU.min)

            prep(0)
            for c in range(CPC):
                if c + 1 < CPC:
                    prep(c + 1)
                pairwise(c)

            # ---- finals ----
            if DEBUG:
                nc.sync.dma_start(dpmin_d[:], pmin[:])
            red = const.tile([128, 2 * CPC], FP32R)
            with nc.allow_low_precision(reason="fp32r is fp32-width"):
                nc.vector.tensor_reduce(
                    out=red[:],
                    in_=pmin[:].rearrange("p (g k) -> p g k", k=JT),
                    axis=AX.X, op=ALU.add)
            psf = psx_p.tile([1, 2 * CPC], FP32, tag="xf")
            nc.tensor.matmul(psf[:], onesc[:, 0:1], red[:], start=True,
                             stop=True)
            outrow = const.tile([1, 2 * CPC], FP32)
            nc.vector.tensor_tensor(
                out=outrow[:], in0=psf[:], in1=finsc[:], op=ALU.mult)
            nc.sync.dma_start(out_d[:], outrow[:])

    nc.compile()
    return nc


def _get_nc():
    if "nc" not in _CACHE:
        _CACHE["nc"] = _build_bass()
    return _CACHE["nc"]


def _pow2_below(x):
    return 2.0 ** math.floor(math.log2(x))


def kernel(protos1, protos2, W, b, num_classes):
    import ml_dtypes
    from concourse.bass_utils import run_bass_kernel_spmd

    nc_classes = int(num_classes)
    assert nc_classes == C and protos1.shape == (P, D)

    protos1 = np.ascontiguousarray(protos1, dtype=np.float32)
    protos2 = np.ascontiguousarray(protos2, dtype=np.float32)
    W = np.asarray(W, dtype=np.float32)
    b = np.asarray(b, dtype=np.float32)

    # transform matrices (lhsT [d, d']) with the -2 scale folded in
    V = np.linalg.inv(W.T.astype(np.float64)).astype(np.float32)
    V2 = (-2.0 * V).astype(np.float32)
    Wt2 = (-2.0 * W.T).astype(np.float32)
    bias0 = (2.0 * (b.astype(np.float64) @ V.astype(np.float64))).astype(
        np.float32)                      # dir0: +2*(b@V)
    bias1 = (-2.0 * b).astype(np.float32)

    # fp8 scales (powers of two, bounded to e4m3 range 240)
    mx = max(np.abs(protos1).max(), np.abs(protos2).max())
    sx = _pow2_below(224.0 / mx)
    n1 = np.sqrt((protos1.astype(np.float64) ** 2).sum(1))
    n2b = np.sqrt(((protos2.astype(np.float64) - b) ** 2).sum(1))
    colV = np.sqrt((V.astype(np.float64) ** 2).sum(0)).max()
    colW = np.sqrt((W.T.astype(np.float64) ** 2).sum(0)).max()
    B0 = 2.0 * n2b.max() * colV
    B1 = 2.0 * (n1.max() * colW + np.abs(b).max())
    sy0 = _pow2_below(224.0 / B0)
    sy1 = _pow2_below(224.0 / B1)

    # d-major class-sliced tables: (C, NPC, D) -> (C, D, NPC) -> (C,2,128,NPC)
    def dmajor(p):
        pc = p.reshape(NPC, C, D).transpose(1, 2, 0)      # (C, D, NPC)
        return np.ascontiguousarray(pc).reshape(C, 2, 128, NPC)

    p1t = dmajor(protos1)
    p2t = dmajor(protos2)
    q1t = (p1t * np.float32(sx)).astype(ml_dtypes.float8_e4m3)
    q2t = (p2t * np.float32(sx)).astype(ml_dtypes.float8_e4m3)

    # host xs: mean_i |x_i|^2 per class from the quantized tables
    def xsm_of(q):
        f = q.astype(np.float32).astype(np.float64) / sx
        return (f ** 2).sum(axis=(1, 2)).mean(axis=1)     # (C,)

    xsm = np.stack([xsm_of(q1t), xsm_of(q2t)]).astype(np.float64)  # (2, C)

    mats = np.stack([
        np.stack([V2[0:128, :], V2[128:256, :]]),
        np.stack([Wt2[0:128, :], Wt2[128:256, :]]),
    ]).astype(np.float32)                                 # [2, 2, 128, 256]
    biases = np.stack([
        np.concatenate([(bias0 * sy0).reshape(2, 128).T,
                        np.full((128, 1), sy0, np.float32)], axis=1),
        np.concatenate([(bias1 * sy1).reshape(2, 128).T,
                        np.full((128, 1), sy1, np.float32)], axis=1),
    ]).astype(np.float32)                                 # [2, 128, 3]
    onesc = np.stack([
        np.ones(128, np.float32),
        np.full(128, sx / (4.0 * sy0), np.float32),
        np.full(128, sx / (4.0 * sy1), np.float32),
    ], axis=1).astype(np.float32)                         # [128, 3]
    idb = np.eye(128, dtype=np.float32).astype(ml_dtypes.bfloat16)
    finsc = np.concatenate([
        np.full(CPC, 1.0 / (NPC * sx * sy0), np.float64),
        np.full(CPC, 1.0 / (NPC * sx * sy1), np.float64),
    ]).astype(np.float32).reshape(1, 2 * CPC)

    in_maps = []
    for core in range(N_CORES):
        sl = slice(core * CPC, (core + 1) * CPC)
        in_maps.append({
            "p1t": np.ascontiguousarray(p1t[sl]),
            "p2t": np.ascontiguousarray(p2t[sl]),
            "q1t": np.ascontiguousarray(q1t[sl]),
            "q2t": np.ascontiguousarray(q2t[sl]),
            "mats": mats,
            "biases": biases,
            "onesc": onesc,
            "idb": idb,
            "finsc": finsc,
            "onef": np.ones((1, 2), dtype=np.float32),
        })

    nc = _get_nc()
    res = run_bass_kernel_spmd(nc, in_maps, core_ids=list(range(N_CORES)))
    _CACHE["last_result"] = res

    out = np.zeros((2, C), dtype=np.float64)
    for core in range(N_CORES):
        row = res.results[core]["out"].reshape(2, CPC).astype(np.float64)
        sl = slice(core * CPC, (core + 1) * CPC)
        out[0, sl] = row[0] + xsm[0, sl]
        out[1, sl] = row[1] + xsm[1, sl]
    return out.astype(np.float32)


# revision 15
# speedup vs baseline: 1.1741x; 1.0105x over previous
"""Trainium2 Bass kernel for nn_ProtoCycleModel (retrieval_knn), v2.

Problem: P=65536 prototypes, C=64 classes, D=256.
Per class c (rows c::64 of each table, n=1024):
    loss_src[c] = mean_i min_j ||p1_c[i] - inv(W.T)@(p2_c[j]-b)||^2
    loss_tgt[c] = mean_i min_j ||p2_c[i] - (W.T@p1_c[j]+b)||^2
Output: (2, 64) fp32.  Sharding: 8 classes per core.

Design ("flipped layout"):
  - Host sends tables d-major (C, 2, 128, NPC) as fp32(r) AND fp8e4
    (scaled by power-of-2 sx); host also precomputes mean|x|^2 per class
    (added to the device result at the end, like the inv(W) prep).
  - Transform y' = M@x + b on PE in fp32r -> yt8 (fp8, scale sy_dr) via ACT.
  - sq = yt8^2 (ACT or Pool), ysrow = ones^T sq (PE, value sx/(4 sy)) ->
    ys scatter-DMA'd from the [1,1024] psum row into [128, 8] columns.
  - Pairwise G'[j%128, i] = sum_d qx[d,i] * yt8[d,j]: ONE fp8 DoubleRow
    matmul per 128-j tile (K=256 in one pass, 0.5 cycles/row).
  - j sits on PSUM partitions, so +|y'|^2 is a per-partition scalar:
    DVE scalar_tensor_tensor fuses (G + ys) and running min across
    j-tiles in the single required PSUM pass; ACT-assigned class-dirs
    use activation(bias=ys_col) copies merged by Pool tensor_tensor min.
  - Finish per class-dir: 8 PE transposes of the [128,1024] bf16 running
    min -> psum [128, 8, 128], one DVE min-reduce -> pmin columns;
    final: add-reduce, ones-matmul cross-partition sum, scale, DMA out.
"""

import math
import os

import numpy as np

P, C, D = 65536, 64, 256
N_CORES = 8
CPC = C // N_CORES          # classes per core = 8
NPC = P // C                # prototypes per class = 1024
JT = NPC // 128             # j-tiles per class-dir = 8

# dir1 classes whose stream runs on ACT+Pool instead of DVE
N_ACT_CDS = int(os.environ.get("K_ACT_CDS", "10"))
# sq on Pool for dir0 (else ACT)
SQ_POOL_D0 = os.environ.get("K_SQ_POOL_D0", "0") == "1"

_CACHE = {}


def _build_bass():
    from concourse import bacc
    import concourse.tile as tile
    from concourse import mybir

    FP32 = mybir.dt.float32
    FP32R = mybir.dt.float32r
    BF16 = mybir.dt.bfloat16
    FP8 = mybir.dt.float8e4
    AF = mybir.ActivationFunctionType
    ALU = mybir.AluOpType
    AX = mybir.AxisListType
    PM = mybir.MatmulPerfMode

    nc = bacc.Bacc(None, target_bir_lowering=False)

    p1t_d = nc.dram_tensor("p1t", [CPC, 2, 128, NPC], FP32R, kind="ExternalInput")
    p2t_d = nc.dram_tensor("p2t", [CPC, 2, 128, NPC], FP32R, kind="ExternalInput")
    q1t_d = nc.dram_tensor("q1t", [CPC, 2, 128, NPC], FP8, kind="ExternalInput")
    q2t_d = nc.dram_tensor("q2t", [CPC, 2, 128, NPC], FP8, kind="ExternalInput")
    # mats[dir][dc]: [128, 256] fp32r, lhsT [d, d'] with -2 folded in
    mats_d = nc.dram_tensor("mats", [2, 2, 128, D], FP32R, kind="ExternalInput")
    # biases[dir][dcp] per-partition: sy_dr * bias_raw_dr
    bias_d = nc.dram_tensor("biases", [2, 128, 6], FP32, kind="ExternalInput")
    # consts cols: 0 = ones 1.0, 1..2 = sx/(4*sy_dr)
    ones_d = nc.dram_tensor("onesc", [128, 3], FP32R, kind="ExternalInput")
    idb_d = nc.dram_tensor("idb", [128, 128], BF16, kind="ExternalInput")
    finsc_d = nc.dram_tensor("finsc", [1, 2 * CPC], FP32, kind="ExternalInput")
    onef_d = nc.dram_tensor("onef", [1, 2], FP32, kind="ExternalInput")
    out_d = nc.dram_tensor("out", [1, 2 * CPC], FP32, kind="ExternalOutput")
    DEBUG = os.environ.get("K_DEBUG", "0") == "1"
    if DEBUG:
        dpmin_d = nc.dram_tensor("dpmin", [128, 2 * CPC * JT], FP32,
                                 kind="ExternalOutput")
        dysc_d = nc.dram_tensor("dysc", [2, 128, JT], FP32,
                                kind="ExternalOutput")
        dyt8_d = nc.dram_tensor("dyt8", [128, 2, NPC], FP32,
                                kind="ExternalOutput")

    act_cds = {(c, 1) for c in range(min(N_ACT_CDS, 8))} | {
        (c, 0) for c in range(max(0, N_ACT_CDS - 8))}

    with tile.TileContext(nc) as tc:
        with (
            tc.tile_pool(name="const", bufs=1) as const,
            tc.tile_pool(name="xt", bufs=2) as xt_p,
            tc.tile_pool(name="qx", bufs=2) as qx_p,
            tc.tile_pool(name="yt", bufs=2) as yt_p,
            tc.tile_pool(name="sq", bufs=2) as sq_p,
            tc.tile_pool(name="ysc", bufs=2) as ysc_p,
            tc.tile_pool(name="run", bufs=3) as run_p,
            tc.tile_pool(name="gb", bufs=4) as gb_p,
            tc.tile_pool(name="mg", bufs=3) as mg_p,
            tc.tile_pool(name="psg", bufs=2, space="PSUM") as psg_p,
            tc.tile_pool(name="psx", bufs=2, space="PSUM") as psx_p,
        ):
            # ---- constants ----
            mats = const.tile([128, 2, 2, D], FP32R)
            nc.sync.dma_start(mats[:], mats_d[:].rearrange("a b p d -> p a b d"))
            biases = const.tile([128, 2, 6], FP32)
            nc.sync.dma_start(biases[:], bias_d[:].rearrange("a p c -> p a c"))
            onesc = const.tile([128, 3], FP32R)
            nc.sync.dma_start(onesc[:], ones_d[:])
            idb = const.tile([128, 128], BF16)
            nc.sync.dma_start(idb[:], idb_d[:])
            finsc = const.tile([1, 2 * CPC], FP32)
            nc.sync.dma_start(finsc[:], finsc_d[:])
            onef = const.tile([1, 2], FP32)
            nc.sync.dma_start(onef[:], onef_d[:])

            pmin = const.tile([128, 2 * CPC * JT], FP32)  # col = dr*64+c*8+ib

            state = {}

            def prep(c):
                xts, qxs = [], []
                for t, (src_d, qsrc_d) in enumerate(
                    ((p1t_d, q1t_d), (p2t_d, q2t_d))
                ):
                    xt = xt_p.tile([128, 2, NPC], FP32R, tag=f"xt{t}")
                    nc.sync.dma_start(
                        xt[:], src_d[c].rearrange("a p j -> p a j"))
                    qx = qx_p.tile([128, 2, NPC], FP8, tag=f"qx{t}")
                    nc.sync.dma_start(
                        qx[:], qsrc_d[c].rearrange("a p j -> p a j"))
                    xts.append(xt)
                    qxs.append(qx)

                yt8s, yscs = [], []
                for dr in range(2):
                    ysrc = xts[1 - dr]   # dir0: y from p2; dir1: y from p1
                    yt8 = yt_p.tile([128, 2, NPC], FP8, tag=f"yt{dr}")
                    sq = sq_p.tile([128, 2, NPC], FP32R, tag=f"sq{dr}")
                    for dcp in range(2):
                        pstf = psx_p.tile([128, NPC], FP32, tag="xf")
                        for dc in range(2):
                            for ih in range(2):
                                nc.tensor.matmul(
                                    pstf[:, ih * 512:(ih + 1) * 512],
                                    mats[:, dr, dc, dcp * 128:(dcp + 1) * 128],
                                    ysrc[:, dc, ih * 512:(ih + 1) * 512],
                                    start=(dc == 0), stop=(dc == 1),
                                )
                        nc.scalar.activation(
                            yt8[:, dcp, :], pstf[:], AF.Identity,
                            bias=biases[:, dr, dcp:dcp + 1],
                            scale=biases[:, dr, 2:3])
                        nc.scalar.activation(
                            sq[:, dcp, :], pstf[:], AF.Square,
                            bias=biases[:, dr, 3 + dcp:4 + dcp],
                            scale=biases[:, dr, 5:6])
                    ysp = psx_p.tile([1, NPC], FP32, tag="xf")
                    for jh in range(2):
                        for dcp in range(2):
                            nc.tensor.matmul(
                                ysp[:, jh * 512:(jh + 1) * 512],
                                onesc[:, 1 + dr:2 + dr],
                                sq[:, dcp, jh * 512:(jh + 1) * 512],
                                start=(dcp == 0), stop=(dcp == 1),
                            )
                    ysr = ysc_p.tile([1, NPC], FP32, tag=f"ysr{dr}")
                    nc.scalar.copy(ysr[:], ysp[:])
                    ysp2 = psx_p.tile([128, JT], FP32, tag="xf")
                    for jt in range(JT):
                        nc.tensor.matmul(
                            ysp2[:, jt:jt + 1],
                            ysr[:, jt * 128:(jt + 1) * 128],
                            onef[0:1, 0:1],
                            start=True, stop=True,
                        )
                    ysc = ysc_p.tile([128, JT], FP32, tag=f"ys{dr}")
                    nc.vector.tensor_copy(ysc[:], ysp2[:])
                    if DEBUG and c == 0:
                        nc.sync.dma_start(dysc_d[dr], ysc[:])
                        if dr == 0:
                            dy = const.tile([128, 2, NPC], FP32, name="dy")
                            nc.vector.tensor_copy(dy[:], yt8[:])
                            nc.sync.dma_start(dyt8_d[:], dy[:])
                    yt8s.append(yt8)
                    yscs.append(ysc)
                state[c] = (qxs, yt8s, yscs)

            def pairwise(c):
                qxs, yt8s, yscs = state.pop(c)
                runs = [None, None]
                for jt in range(JT):
                    for dr in range(2):
                        g = psg_p.tile([128, NPC], FP32, tag="g")
                        for jh in range(2):
                            nc.tensor.matmul(
                                g[:, jh * 512:(jh + 1) * 512],
                                yt8s[dr][:, :, jt * 128:(jt + 1) * 128],
                                qxs[dr][:, :, jh * 512:(jh + 1) * 512],
                                start=True, stop=True,
                                perf_mode=PM.DoubleRow,
                            )
                        ys_col = yscs[dr][:, jt:jt + 1]
                        if (c, dr) in act_cds:
                            gb = gb_p.tile([128, NPC], BF16, tag="gb")
                            nc.scalar.activation(
                                gb[:], g[:], AF.Identity, bias=ys_col,
                                scale=1.0)
                            if jt == 0:
                                runs[dr] = gb
                            else:
                                mg = mg_p.tile([128, NPC], BF16, tag="mg")
                                nc.vector.tensor_tensor(
                                    out=mg[:], in0=runs[dr][:], in1=gb[:],
                                    op=ALU.min)
                                runs[dr] = mg
                        else:
                            nrun = run_p.tile([128, NPC], BF16, tag="run")
                            if jt == 0:
                                nc.vector.tensor_scalar(
                                    out=nrun[:], in0=g[:], scalar1=ys_col,
                                    scalar2=None, op0=ALU.add)
                            else:
                                nc.vector.scalar_tensor_tensor(
                                    out=nrun[:], in0=g[:], scalar=ys_col,
                                    in1=runs[dr][:], op0=ALU.add,
                                    op1=ALU.min)
                            runs[dr] = nrun
                for dr in range(2):
                    ft = psg_p.tile([128, JT, 128], BF16, tag="g")
                    for ib in range(JT):
                        nc.tensor.transpose(
                            ft[:, ib, :],
                            runs[dr][:, ib * 128:(ib + 1) * 128],
                            idb[:],
                        )
                    base = dr * 64 + c * 8
                    nc.vector.tensor_reduce(
                        out=pmin[:, base:base + JT], in_=ft[:],
                        axis=AX.X, op=ALU.min)

            prep(0)
            for c in range(CPC):
                if c + 1 < CPC:
                    prep(c + 1)
                pairwise(c)

            # ---- finals ----
            if DEBUG:
                nc.sync.dma_start(dpmin_d[:], pmin[:])
            red = const.tile([128, 2 * CPC], FP32R)
            with nc.allow_low_precision(reason="fp32r is fp32-width"):
                nc.vector.tensor_reduce(
                    out=red[:],
                    in_=pmin[:].rearrange("p (g k) -> p g k", k=JT),
                    axis=AX.X, op=ALU.add)
            psf = psx_p.tile([1, 2 * CPC], FP32, tag="xf")
            nc.tensor.matmul(psf[:], onesc[:, 0:1], red[:], start=True,
                             stop=True)
            outrow = const.tile([1, 2 * CPC], FP32)
            nc.vector.tensor_tensor(
                out=outrow[:], in0=psf[:], in1=finsc[:], op=ALU.mult)
            nc.sync.dma_start(out_d[:], outrow[:])

    nc.compile()
    return nc


def _get_nc():
    if "nc" not in _CACHE:
        _CACHE["nc"] = _build_bass()
    return _CACHE["nc"]


def _pow2_below(x):
    return 2.0 ** math.floor(math.log2(x))


def kernel(protos1, protos2, W, b, num_classes):
    import ml_dtypes
    from concourse.bass_utils import run_bass_kernel_spmd

    nc_classes = int(num_classes)
    assert nc_classes == C and protos1.shape == (P, D)

    protos1 = np.ascontiguousarray(protos1, dtype=np.float32)
    protos2 = np.ascontiguousarray(protos2, dtype=np.float32)
    W = np.asarray(W, dtype=np.float32)
    b = np.asarray(b, dtype=np.float32)

    # transform matrices (lhsT [d, d']) with the -2 scale folded in
    V = np.linalg.inv(W.T.astype(np.float64)).astype(np.float32)
    V2 = (-2.0 * V).astype(np.float32)
    Wt2 = (-2.0 * W.T).astype(np.float32)
    bias0 = (2.0 * (b.astype(np.float64) @ V.astype(np.float64))).astype(
        np.float32)                      # dir0: +2*(b@V)
    bias1 = (-2.0 * b).astype(np.float32)

    # fp8 scales (powers of two, bounded to e4m3 range 240)
    mx = max(np.abs(protos1).max(), np.abs(protos2).max())
    sx = _pow2_below(224.0 / mx)
    n1 = np.sqrt((protos1.astype(np.float64) ** 2).sum(1))
    n2b = np.sqrt(((protos2.astype(np.float64) - b) ** 2).sum(1))
    colV = np.sqrt((V.astype(np.float64) ** 2).sum(0)).max()
    colW = np.sqrt((W.T.astype(np.float64) ** 2).sum(0)).max()
    B0 = 2.0 * n2b.max() * colV
    B1 = 2.0 * (n1.max() * colW + np.abs(b).max())
    sy0 = _pow2_below(224.0 / B0)
    sy1 = _pow2_below(224.0 / B1)

    # d-major class-sliced tables: (C, NPC, D) -> (C, D, NPC) -> (C,2,128,NPC)
    def dmajor(p):
        pc = p.reshape(NPC, C, D).transpose(1, 2, 0)      # (C, D, NPC)
        return np.ascontiguousarray(pc).reshape(C, 2, 128, NPC)

    p1t = dmajor(protos1)
    p2t = dmajor(protos2)
    q1t = (p1t * np.float32(sx)).astype(ml_dtypes.float8_e4m3)
    q2t = (p2t * np.float32(sx)).astype(ml_dtypes.float8_e4m3)

    # host xs: mean_i |x_i|^2 per class from the quantized tables
    def xsm_of(q):
        f = q.astype(np.float32).astype(np.float64) / sx
        return (f ** 2).sum(axis=(1, 2)).mean(axis=1)     # (C,)

    xsm = np.stack([xsm_of(q1t), xsm_of(q2t)]).astype(np.float64)  # (2, C)

    mats = np.stack([
        np.stack([V2[0:128, :], V2[128:256, :]]),
        np.stack([Wt2[0:128, :], Wt2[128:256, :]]),
    ]).astype(np.float32)                                 # [2, 2, 128, 256]
    ssq0 = math.sqrt(sx * sy0) / 2.0
    ssq1 = math.sqrt(sx * sy1) / 2.0
    biases = np.stack([
        np.concatenate([(bias0 * sy0).reshape(2, 128).T,
                        np.full((128, 1), sy0, np.float32),
                        (bias0 * ssq0).reshape(2, 128).T,
                        np.full((128, 1), ssq0, np.float32)], axis=1),
        np.concatenate([(bias1 * sy1).reshape(2, 128).T,
                        np.full((128, 1), sy1, np.float32),
                        (bias1 * ssq1).reshape(2, 128).T,
                        np.full((128, 1), ssq1, np.float32)], axis=1),
    ]).astype(np.float32)                                 # [2, 128, 6]
    onesc = np.ones((128, 3), dtype=np.float32)
    idb = np.eye(128, dtype=np.float32).astype(ml_dtypes.bfloat16)
    finsc = np.concatenate([
        np.full(CPC, 1.0 / (NPC * sx * sy0), np.float64),
        np.full(CPC, 1.0 / (NPC * sx * sy1), np.float64),
    ]).astype(np.float32).reshape(1, 2 * CPC)

    in_maps = []
    for core in range(N_CORES):
        sl = slice(core * CPC, (core + 1) * CPC)
        in_maps.append({
            "p1t": np.ascontiguousarray(p1t[sl]),
            "p2t": np.ascontiguousarray(p2t[sl]),
            "q1t": np.ascontiguousarray(q1t[sl]),
            "q2t": np.ascontiguousarray(q2t[sl]),
            "mats": mats,
            "biases": biases,
            "onesc": onesc,
            "idb": idb,
            "finsc": finsc,
            "onef": np.ones((1, 2), dtype=np.float32),
        })

    nc = _get_nc()
    res = run_bass_kernel_spmd(nc, in_maps, core_ids=list(range(N_CORES)))
    _CACHE["last_result"] = res

    out = np.zeros((2, C), dtype=np.float64)
    for core in range(N_CORES):
        row = res.results[core]["out"].reshape(2, CPC).astype(np.float64)
        sl = slice(core * CPC, (core + 1) * CPC)
        out[0, sl] = row[0] + xsm[0, sl]
        out[1, sl] = row[1] + xsm[1, sl]
    return out.astype(np.float32)


# revision 17
# speedup vs baseline: 1.1751x; 1.0009x over previous
"""Trainium2 Bass kernel for nn_ProtoCycleModel (retrieval_knn), v2.

Problem: P=65536 prototypes, C=64 classes, D=256.
Per class c (rows c::64 of each table, n=1024):
    loss_src[c] = mean_i min_j ||p1_c[i] - inv(W.T)@(p2_c[j]-b)||^2
    loss_tgt[c] = mean_i min_j ||p2_c[i] - (W.T@p1_c[j]+b)||^2
Output: (2, 64) fp32.  Sharding: 8 classes per core.

Design ("flipped layout"):
  - Host sends tables d-major (C, 2, 128, NPC) as fp32(r) AND fp8e4
    (scaled by power-of-2 sx); host also precomputes mean|x|^2 per class
    (added to the device result at the end, like the inv(W) prep).
  - Transform y' = M@x + b on PE in fp32r -> yt8 (fp8, scale sy_dr) via ACT.
  - sq = yt8^2 (ACT or Pool), ysrow = ones^T sq (PE, value sx/(4 sy)) ->
    ys scatter-DMA'd from the [1,1024] psum row into [128, 8] columns.
  - Pairwise G'[j%128, i] = sum_d qx[d,i] * yt8[d,j]: ONE fp8 DoubleRow
    matmul per 128-j tile (K=256 in one pass, 0.5 cycles/row).
  - j sits on PSUM partitions, so +|y'|^2 is a per-partition scalar:
    DVE scalar_tensor_tensor fuses (G + ys) and running min across
    j-tiles in the single required PSUM pass; ACT-assigned class-dirs
    use activation(bias=ys_col) copies merged by Pool tensor_tensor min.
  - Finish per class-dir: 8 PE transposes of the [128,1024] bf16 running
    min -> psum [128, 8, 128], one DVE min-reduce -> pmin columns;
    final: add-reduce, ones-matmul cross-partition sum, scale, DMA out.
"""

import math
import os

import numpy as np

P, C, D = 65536, 64, 256
N_CORES = 8
CPC = C // N_CORES          # classes per core = 8
NPC = P // C                # prototypes per class = 1024
JT = NPC // 128             # j-tiles per class-dir = 8

# ACT-streamed j-tiles per class-dir (rest go through the DVE stt chain)
ACT_TILES = int(os.environ.get("K_ACT_TILES", "4"))

_CACHE = {}


def _build_bass():
    from concourse import bacc
    import concourse.tile as tile
    from concourse import mybir

    FP32 = mybir.dt.float32
    FP32R = mybir.dt.float32r
    BF16 = mybir.dt.bfloat16
    FP8 = mybir.dt.float8e4
    AF = mybir.ActivationFunctionType
    ALU = mybir.AluOpType
    AX = mybir.AxisListType
    PM = mybir.MatmulPerfMode

    nc = bacc.Bacc(None, target_bir_lowering=False)

    p1t_d = nc.dram_tensor("p1t", [CPC, 2, 128, NPC], FP32R, kind="ExternalInput")
    p2t_d = nc.dram_tensor("p2t", [CPC, 2, 128, NPC], FP32R, kind="ExternalInput")
    q1t_d = nc.dram_tensor("q1t", [CPC, 2, 128, NPC], FP8, kind="ExternalInput")
    q2t_d = nc.dram_tensor("q2t", [CPC, 2, 128, NPC], FP8, kind="ExternalInput")
    # mats[dir][dc]: [128, 256] fp32r, lhsT [d, d'] with -2 folded in
    mats_d = nc.dram_tensor("mats", [2, 2, 128, D], FP32R, kind="ExternalInput")
    # biases[dir][dcp] per-partition: sy_dr * bias_raw_dr
    bias_d = nc.dram_tensor("biases", [2, 128, 6], FP32, kind="ExternalInput")
    # consts cols: 0 = ones 1.0, 1..2 = sx/(4*sy_dr)
    ones_d = nc.dram_tensor("onesc", [128, 3], FP32, kind="ExternalInput")
    idb_d = nc.dram_tensor("idb", [128, 128], BF16, kind="ExternalInput")
    finsc_d = nc.dram_tensor("finsc", [1, 2 * CPC], FP32, kind="ExternalInput")
    out_d = nc.dram_tensor("out", [1, 2 * CPC], FP32, kind="ExternalOutput")
    DEBUG = os.environ.get("K_DEBUG", "0") == "1"
    if DEBUG:
        dpmin_d = nc.dram_tensor("dpmin", [128, 2 * CPC * JT], FP32,
                                 kind="ExternalOutput")
        dysc_d = nc.dram_tensor("dysc", [2, 128, JT], FP32,
                                kind="ExternalOutput")
        dyt8_d = nc.dram_tensor("dyt8", [128, 2, NPC], FP32,
                                kind="ExternalOutput")



    with tile.TileContext(nc) as tc:
        with (
            tc.tile_pool(name="const", bufs=1) as const,
            tc.tile_pool(name="xt", bufs=2) as xt_p,
            tc.tile_pool(name="qx", bufs=2) as qx_p,
            tc.tile_pool(name="yt", bufs=2) as yt_p,
            tc.tile_pool(name="sq", bufs=2) as sq_p,
            tc.tile_pool(name="ysc", bufs=2) as ysc_p,
            tc.tile_pool(name="run", bufs=3) as run_p,
            tc.tile_pool(name="gb", bufs=4) as gb_p,
            tc.tile_pool(name="mg", bufs=3) as mg_p,
            tc.tile_pool(name="psg", bufs=2, space="PSUM") as psg_p,
            tc.tile_pool(name="psx", bufs=1, space="PSUM") as psx_p,
        ):
            # ---- constants ----
            mats = const.tile([128, 2, 2, D], FP32R)
            nc.sync.dma_start(mats[:], mats_d[:].rearrange("a b p d -> p a b d"))
            biases = const.tile([128, 2, 6], FP32)
            nc.sync.dma_start(biases[:], bias_d[:].rearrange("a p c -> p a c"))
            onesc = const.tile([128, 3], FP32)
            nc.sync.dma_start(onesc[:], ones_d[:])
            idb = const.tile([128, 128], BF16)
            nc.sync.dma_start(idb[:], idb_d[:])
            finsc = const.tile([1, 2 * CPC], FP32)
            nc.sync.dma_start(finsc[:], finsc_d[:])

            pmin = const.tile([128, 2 * CPC * JT], FP32)  # col = dr*64+c*8+ib

            state = {}

            def prep(c):
                xts, qxs = [], []
                for t, (src_d, qsrc_d) in enumerate(
                    ((p1t_d, q1t_d), (p2t_d, q2t_d))
                ):
                    xt = xt_p.tile([128, 2, NPC], FP32R, tag=f"xt{t}")
                    nc.sync.dma_start(
                        xt[:], src_d[c].rearrange("a p j -> p a j"))
                    qx = qx_p.tile([128, 2, NPC], FP8, tag=f"qx{t}")
                    nc.sync.dma_start(
                        qx[:], qsrc_d[c].rearrange("a p j -> p a j"))
                    xts.append(xt)
                    qxs.append(qx)

                yt8s, yscs = [], []
                for dr in range(2):
                    ysrc = xts[1 - dr]   # dir0: y from p2; dir1: y from p1
                    yt8 = yt_p.tile([128, 2, NPC], FP8, tag=f"yt{dr}")
                    sq = sq_p.tile([128, 2, NPC], FP32, tag=f"sq{dr}")
                    for dcp in range(2):
                        pstf = psx_p.tile([128, NPC], FP32, tag="xf")
                        for dc in range(2):
                            for ih in range(2):
                                nc.tensor.matmul(
                                    pstf[:, ih * 512:(ih + 1) * 512],
                                    mats[:, dr, dc, dcp * 128:(dcp + 1) * 128],
                                    ysrc[:, dc, ih * 512:(ih + 1) * 512],
                                    start=(dc == 0), stop=(dc == 1),
                                )
                        nc.scalar.activation(
                            yt8[:, dcp, :], pstf[:], AF.Identity,
                            bias=biases[:, dr, dcp:dcp + 1],
                            scale=biases[:, dr, 2:3])
                        nc.scalar.activation(
                            sq[:, dcp, :], pstf[:], AF.Square,
                            bias=biases[:, dr, 3 + dcp:4 + dcp],
                            scale=biases[:, dr, 5:6])
                    ysp2 = psg_p.tile([128, JT], FP32, tag="fy")
                    for jt in range(JT):
                        for dcp in range(2):
                            nc.tensor.matmul(
                                ysp2[:, jt:jt + 1],
                                sq[:, dcp, jt * 128:(jt + 1) * 128],
                                onesc[:, 0:1],
                                start=(dcp == 0), stop=(dcp == 1),
                            )
                    ysc = ysc_p.tile([128, JT], FP32, tag=f"ys{dr}")
                    nc.vector.tensor_copy(ysc[:], ysp2[:])
                    if DEBUG and c == 0:
                        nc.sync.dma_start(dysc_d[dr], ysc[:])
                        if dr == 0:
                            dy = const.tile([128, 2, NPC], FP32, name="dy")
                            nc.vector.tensor_copy(dy[:], yt8[:])
                            nc.sync.dma_start(dyt8_d[:], dy[:])
                    yt8s.append(yt8)
                    yscs.append(ysc)
                state[c] = (qxs, yt8s, yscs)

            def pairwise(c):
                qxs, yt8s, yscs = state.pop(c)
                runs = [None, None]
                for jt in range(JT):
                    for dr in range(2):
                        g = psg_p.tile([128, NPC], FP32, tag="g")
                        for jh in range(2):
                            nc.tensor.matmul(
                                g[:, jh * 512:(jh + 1) * 512],
                                yt8s[dr][:, :, jt * 128:(jt + 1) * 128],
                                qxs[dr][:, :, jh * 512:(jh + 1) * 512],
                                start=True, stop=True,
                                perf_mode=PM.DoubleRow,
                            )
                        ys_col = yscs[dr][:, jt:jt + 1]
                        if jt < ACT_TILES:
                            # ACT stream: copy+bias to bf16, DVE TT-min merge
                            gb = gb_p.tile([128, NPC], BF16, tag="gb")
                            nc.scalar.activation(
                                gb[:], g[:], AF.Identity, bias=ys_col,
                                scale=1.0)
                            if jt == 0:
                                runs[dr] = gb
                            else:
                                mg = mg_p.tile([128, NPC], BF16, tag="mg")
                                nc.vector.tensor_tensor(
                                    out=mg[:], in0=runs[dr][:], in1=gb[:],
                                    op=ALU.min)
                                runs[dr] = mg
                        else:
                            # DVE stt chain (absorbs the ACT merge as in1)
                            nrun = run_p.tile([128, NPC], BF16, tag="run")
                            if runs[dr] is None:
                                nc.vector.tensor_scalar(
                                    out=nrun[:], in0=g[:], scalar1=ys_col,
                                    scalar2=None, op0=ALU.add)
                            else:
                                nc.vector.scalar_tensor_tensor(
                                    out=nrun[:], in0=g[:], scalar=ys_col,
                                    in1=runs[dr][:], op0=ALU.add,
                                    op1=ALU.min)
                            runs[dr] = nrun
                for dr in range(2):
                    ft = psg_p.tile([128, JT, 128], BF16, tag="fy")
                    for ib in range(JT):
                        nc.tensor.transpose(
                            ft[:, ib, :],
                            runs[dr][:, ib * 128:(ib + 1) * 128],
                            idb[:],
                        )
                    base = dr * 64 + c * 8
                    nc.vector.tensor_reduce(
                        out=pmin[:, base:base + JT], in_=ft[:],
                        axis=AX.X, op=ALU.min)

            prep(0)
            for c in range(CPC):
                pairwise(c)
                if c + 1 < CPC:
                    prep(c + 1)

            # ---- finals ----
            if DEBUG:
                nc.sync.dma_start(dpmin_d[:], pmin[:])
            red = const.tile([128, 2 * CPC], FP32)
            nc.vector.tensor_reduce(
                out=red[:],
                in_=pmin[:].rearrange("p (g k) -> p g k", k=JT),
                axis=AX.X, op=ALU.add)
            psf = psx_p.tile([1, 2 * CPC], FP32, tag="xf")
            nc.tensor.matmul(psf[:], onesc[:, 0:1], red[:], start=True,
                             stop=True)
            outrow = const.tile([1, 2 * CPC], FP32)
            nc.vector.tensor_tensor(
                out=outrow[:], in0=psf[:], in1=finsc[:], op=ALU.mult)
            nc.sync.dma_start(out_d[:], outrow[:])

    nc.compile()
    return nc


def _get_nc():
    if "nc" not in _CACHE:
        _CACHE["nc"] = _build_bass()
    return _CACHE["nc"]


def _pow2_below(x):
    return 2.0 ** math.floor(math.log2(x))


def kernel(protos1, protos2, W, b, num_classes):
    import ml_dtypes
    from concourse.bass_utils import run_bass_kernel_spmd

    nc_classes = int(num_classes)
    assert nc_classes == C and protos1.shape == (P, D)

    protos1 = np.ascontiguousarray(protos1, dtype=np.float32)
    protos2 = np.ascontiguousarray(protos2, dtype=np.float32)
    W = np.asarray(W, dtype=np.float32)
    b = np.asarray(b, dtype=np.float32)

    # transform matrices (lhsT [d, d']) with the -2 scale folded in
    V = np.linalg.inv(W.T.astype(np.float64)).astype(np.float32)
    V2 = (-2.0 * V).astype(np.float32)
    Wt2 = (-2.0 * W.T).astype(np.float32)
    bias0 = (2.0 * (b.astype(np.float64) @ V.astype(np.float64))).astype(
        np.float32)                      # dir0: +2*(b@V)
    bias1 = (-2.0 * b).astype(np.float32)

    # fp8 scales (powers of two, bounded to e4m3 range 240)
    mx = max(np.abs(protos1).max(), np.abs(protos2).max())
    sx = _pow2_below(224.0 / mx)
    n1 = np.sqrt((protos1.astype(np.float64) ** 2).sum(1))
    n2b = np.sqrt(((protos2.astype(np.float64) - b) ** 2).sum(1))
    colV = np.sqrt((V.astype(np.float64) ** 2).sum(0)).max()
    colW = np.sqrt((W.T.astype(np.float64) ** 2).sum(0)).max()
    B0 = 2.0 * n2b.max() * colV
    B1 = 2.0 * (n1.max() * colW + np.abs(b).max())
    sy0 = _pow2_below(224.0 / B0)
    sy1 = _pow2_below(224.0 / B1)

    # d-major class-sliced tables: (C, NPC, D) -> (C, D, NPC) -> (C,2,128,NPC)
    def dmajor(p):
        pc = p.reshape(NPC, C, D).transpose(1, 2, 0)      # (C, D, NPC)
        return np.ascontiguousarray(pc).reshape(C, 2, 128, NPC)

    p1t = dmajor(protos1)
    p2t = dmajor(protos2)
    q1t = (p1t * np.float32(sx)).astype(ml_dtypes.float8_e4m3)
    q2t = (p2t * np.float32(sx)).astype(ml_dtypes.float8_e4m3)

    # host xs: mean_i |x_i|^2 per class from the quantized tables
    def xsm_of(q):
        f = q.astype(np.float32).astype(np.float64) / sx
        return (f ** 2).sum(axis=(1, 2)).mean(axis=1)     # (C,)

    xsm = np.stack([xsm_of(q1t), xsm_of(q2t)]).astype(np.float64)  # (2, C)

    mats = np.stack([
        np.stack([V2[0:128, :], V2[128:256, :]]),
        np.stack([Wt2[0:128, :], Wt2[128:256, :]]),
    ]).astype(np.float32)                                 # [2, 2, 128, 256]
    ssq0 = math.sqrt(sx * sy0) / 2.0
    ssq1 = math.sqrt(sx * sy1) / 2.0
    biases = np.stack([
        np.concatenate([(bias0 * sy0).reshape(2, 128).T,
                        np.full((128, 1), sy0, np.float32),
                        (bias0 * ssq0).reshape(2, 128).T,
                        np.full((128, 1), ssq0, np.float32)], axis=1),
        np.concatenate([(bias1 * sy1).reshape(2, 128).T,
                        np.full((128, 1), sy1, np.float32),
                        (bias1 * ssq1).reshape(2, 128).T,
                        np.full((128, 1), ssq1, np.float32)], axis=1),
    ]).astype(np.float32)                                 # [2, 128, 6]
    onesc = np.ones((128, 3), dtype=np.float32)
    idb = np.eye(128, dtype=np.float32).astype(ml_dtypes.bfloat16)
    finsc = np.concatenate([
        np.full(CPC, 1.0 / (NPC * sx * sy0), np.float64),
        np.full(CPC, 1.0 / (NPC * sx * sy1), np.float64),
    ]).astype(np.float32).reshape(1, 2 * CPC)

    in_maps = []
    for core in range(N_CORES):
        sl = slice(core * CPC, (core + 1) * CPC)
        in_maps.append({
            "p1t": np.ascontiguousarray(p1t[sl]),
            "p2t": np.ascontiguousarray(p2t[sl]),
            "q1t": np.ascontiguousarray(q1t[sl]),
            "q2t": np.ascontiguousarray(q2t[sl]),
            "mats": mats,
            "biases": biases,
            "onesc": onesc,
            "idb": idb,
            "finsc": finsc,
        })

    nc = _get_nc()
    res = run_bass_kernel_spmd(nc, in_maps, core_ids=list(range(N_CORES)))
    _CACHE["last_result"] = res

    out = np.zeros((2, C), dtype=np.float64)
    for core in range(N_CORES):
        row = res.results[core]["out"].reshape(2, CPC).astype(np.float64)
        sl = slice(core * CPC, (core + 1) * CPC)
        out[0, sl] = row[0] + xsm[0, sl]
        out[1, sl] = row[1] + xsm[1, sl]
    return out.astype(np.float32)


# revision 25
# speedup vs baseline: 1.2422x; 1.0571x over previous
"""Trainium2 Bass kernel for nn_ProtoCycleModel (retrieval_knn), v2.

Problem: P=65536 prototypes, C=64 classes, D=256.
Per class c (rows c::64 of each table, n=1024):
    loss_src[c] = mean_i min_j ||p1_c[i] - inv(W.T)@(p2_c[j]-b)||^2
    loss_tgt[c] = mean_i min_j ||p2_c[i] - (W.T@p1_c[j]+b)||^2
Output: (2, 64) fp32.  Sharding: 8 classes per core.

Design ("flipped layout"):
  - Host sends tables d-major (C, 2, 128, NPC) as fp32(r) AND fp8e4
    (scaled by power-of-2 sx); host also precomputes mean|x|^2 per class
    (added to the device result at the end, like the inv(W) prep).
  - Transform y' = M@x + b on PE in fp32r -> yt8 (fp8, scale sy_dr) via ACT.
  - sq = yt8^2 (ACT or Pool), ysrow = ones^T sq (PE, value sx/(4 sy)) ->
    ys scatter-DMA'd from the [1,1024] psum row into [128, 8] columns.
  - Pairwise G'[j%128, i] = sum_d qx[d,i] * yt8[d,j]: ONE fp8 DoubleRow
    matmul per 128-j tile (K=256 in one pass, 0.5 cycles/row).
  - j sits on PSUM partitions, so +|y'|^2 is a per-partition scalar:
    DVE scalar_tensor_tensor fuses (G + ys) and running min across
    j-tiles in the single required PSUM pass; ACT-assigned class-dirs
    use activation(bias=ys_col) copies merged by Pool tensor_tensor min.
  - Finish per class-dir: 8 PE transposes of the [128,1024] bf16 running
    min -> psum [128, 8, 128], one DVE min-reduce -> pmin columns;
    final: add-reduce, ones-matmul cross-partition sum, scale, DMA out.
"""

import math
import os

import numpy as np

P, C, D = 65536, 64, 256
N_CORES = 8
CPC = C // N_CORES          # classes per core = 8
NPC = P // C                # prototypes per class = 1024
JT = NPC // 128             # j-tiles per class-dir = 8

# ACT-streamed j-tiles per class-dir (rest go through the DVE stt chain)
ACT_TILES = int(os.environ.get("K_ACT_TILES", "4"))
SKIP_FINISH = os.environ.get("K_SKIP_FINISH", "0") == "1"
SKIP_STREAM = os.environ.get("K_SKIP_STREAM", "0") == "1"
SKIP_YS = os.environ.get("K_SKIP_YS", "0") == "1"
DEPTH = int(os.environ.get("K_DEPTH", "1"))
BUFS = DEPTH + 1
GBUFS = int(os.environ.get("K_GBUFS", "2"))
XBUFS = int(os.environ.get("K_XBUFS", "2"))

_CACHE = {}


def _build_bass():
    from concourse import bacc
    import concourse.tile as tile
    from concourse import mybir

    FP32 = mybir.dt.float32
    FP32R = mybir.dt.float32r
    BF16 = mybir.dt.bfloat16
    FP8 = mybir.dt.float8e4
    AF = mybir.ActivationFunctionType
    ALU = mybir.AluOpType
    AX = mybir.AxisListType
    PM = mybir.MatmulPerfMode

    nc = bacc.Bacc(None, target_bir_lowering=False)

    p1t_d = nc.dram_tensor("p1t", [CPC, 2, 128, NPC], FP32R, kind="ExternalInput")
    p2t_d = nc.dram_tensor("p2t", [CPC, 2, 128, NPC], FP32R, kind="ExternalInput")
    q1t_d = nc.dram_tensor("q1t", [CPC, 2, 128, NPC], FP8, kind="ExternalInput")
    q2t_d = nc.dram_tensor("q2t", [CPC, 2, 128, NPC], FP8, kind="ExternalInput")
    # mats[dir][dc]: [128, 256] fp32r, lhsT [d, d'] with -2 folded in
    mats_d = nc.dram_tensor("mats", [2, 2, 128, D], FP32R, kind="ExternalInput")
    # biases[dir][dcp] per-partition: sy_dr * bias_raw_dr
    bias_d = nc.dram_tensor("biases", [2, 128, 6], FP32, kind="ExternalInput")
    # consts cols: 0 = ones 1.0, 1..2 = sx/(4*sy_dr)
    ones_d = nc.dram_tensor("onesc", [128, 3], FP32, kind="ExternalInput")
    idb_d = nc.dram_tensor("idb", [128, 128], BF16, kind="ExternalInput")
    finsc_d = nc.dram_tensor("finsc", [1, 2 * CPC], FP32, kind="ExternalInput")
    out_d = nc.dram_tensor("out", [1, 2 * CPC], FP32, kind="ExternalOutput")
    DEBUG = os.environ.get("K_DEBUG", "0") == "1"
    if DEBUG:
        dpmin_d = nc.dram_tensor("dpmin", [128, 2 * CPC * JT], FP32,
                                 kind="ExternalOutput")
        dysc_d = nc.dram_tensor("dysc", [2, 128, JT], FP32,
                                kind="ExternalOutput")
        dyt8_d = nc.dram_tensor("dyt8", [128, 2, NPC], FP32,
                                kind="ExternalOutput")



    with tile.TileContext(nc) as tc:
        with (
            tc.tile_pool(name="const", bufs=1) as const,
            tc.tile_pool(name="xt", bufs=BUFS) as xt_p,
            tc.tile_pool(name="qx", bufs=BUFS) as qx_p,
            tc.tile_pool(name="yt", bufs=BUFS) as yt_p,
            tc.tile_pool(name="sq", bufs=BUFS) as sq_p,
            tc.tile_pool(name="ysc", bufs=BUFS) as ysc_p,
            tc.tile_pool(name="run", bufs=10) as run_p,
            tc.tile_pool(name="gb", bufs=4) as gb_p,
            tc.tile_pool(name="mg", bufs=8) as mg_p,
            tc.tile_pool(name="psg", bufs=GBUFS, space="PSUM") as psg_p,
            tc.tile_pool(name="psx", bufs=XBUFS, space="PSUM") as psx_p,
        ):
            # ---- constants ----
            mats = const.tile([128, 2, 2, D], FP32R)
            nc.sync.dma_start(mats[:], mats_d[:].rearrange("a b p d -> p a b d"))
            biases = const.tile([128, 2, 6], FP32)
            nc.sync.dma_start(biases[:], bias_d[:].rearrange("a p c -> p a c"))
            onesc = const.tile([128, 3], FP32)
            nc.sync.dma_start(onesc[:], ones_d[:])
            idb = const.tile([128, 128], BF16)
            nc.sync.dma_start(idb[:], idb_d[:])
            finsc = const.tile([1, 2 * CPC], FP32)
            nc.sync.dma_start(finsc[:], finsc_d[:])

            pmin = const.tile([128, 2 * CPC * JT], FP32)  # col = dr*64+c*8+ib
            if SKIP_FINISH or SKIP_STREAM:
                nc.vector.memset(pmin[:], 0.0)

            state = {}

            def prep(c):
                xts, qxs = [], []
                for t, (src_d, qsrc_d) in enumerate(
                    ((p1t_d, q1t_d), (p2t_d, q2t_d))
                ):
                    xt = xt_p.tile([128, 2, NPC], FP32R, tag=f"xt{t}")
                    nc.sync.dma_start(
                        xt[:], src_d[c].rearrange("a p j -> p a j"))
                    qx = qx_p.tile([128, 2, NPC], FP8, tag=f"qx{t}")
                    nc.sync.dma_start(
                        qx[:], qsrc_d[c].rearrange("a p j -> p a j"))
                    xts.append(xt)
                    qxs.append(qx)

                yt8s, yscs = [], []
                for dr in range(2):
                    ysrc = xts[1 - dr]   # dir0: y from p2; dir1: y from p1
                    yt8 = yt_p.tile([128, 2, NPC], FP8, tag=f"yt{dr}")
                    sq = (None if SKIP_YS else
                          sq_p.tile([128, 2, NPC], FP32, tag=f"sq{dr}"))
                    for dcp in range(2):
                        pstf = psx_p.tile([128, NPC], FP32, tag="xf")
                        for dc in range(2):
                            for ih in range(2):
                                nc.tensor.matmul(
                                    pstf[:, ih * 512:(ih + 1) * 512],
                                    mats[:, dr, dc, dcp * 128:(dcp + 1) * 128],
                                    ysrc[:, dc, ih * 512:(ih + 1) * 512],
                                    start=(dc == 0), stop=(dc == 1),
                                )
                        if not SKIP_YS:
                            nc.scalar.activation(
                                sq[:, dcp, :], pstf[:], AF.Square,
                                bias=biases[:, dr, 3 + dcp:4 + dcp],
                                scale=biases[:, dr, 5:6])
                        nc.scalar.activation(
                            yt8[:, dcp, :], pstf[:], AF.Identity,
                            bias=biases[:, dr, dcp:dcp + 1],
                            scale=biases[:, dr, 2:3])
                    if SKIP_YS:
                        ysc = ysc_p.tile([128, JT], FP32, tag=f"ys{dr}")
                        nc.vector.memset(ysc[:], 0.0)
                        yt8s.append(yt8)
                        yscs.append(ysc)
                        continue
                    ysp2 = psx_p.tile([128, JT], FP32, tag="xf")
                    for jt in range(JT):
                        for dcp in range(2):
                            nc.tensor.matmul(
                                ysp2[:, jt:jt + 1],
                                sq[:, dcp, jt * 128:(jt + 1) * 128],
                                onesc[:, 0:1],
                                start=(dcp == 0), stop=(dcp == 1),
                            )
                    ysc = ysc_p.tile([128, JT], FP32, tag=f"ys{dr}")
                    nc.vector.tensor_copy(ysc[:], ysp2[:])
                    if DEBUG and c == 0:
                        nc.sync.dma_start(dysc_d[dr], ysc[:])
                        if dr == 0:
                            dy = const.tile([128, 2, NPC], FP32, name="dy")
                            nc.vector.tensor_copy(dy[:], yt8[:])
                            nc.sync.dma_start(dyt8_d[:], dy[:])
                    yt8s.append(yt8)
                    yscs.append(ysc)
                state[c] = (qxs, yt8s, yscs)

            def pairwise(c):
                qxs, yt8s, yscs = state.pop(c)
                runs = [None, None]
                for jt in range(JT):
                    for dr in range(2):
                        g = psg_p.tile([128, NPC], FP32, tag="g")
                        for jh in range(2):
                            nc.tensor.matmul(
                                g[:, jh * 512:(jh + 1) * 512],
                                yt8s[dr][:, :, jt * 128:(jt + 1) * 128],
                                qxs[dr][:, :, jh * 512:(jh + 1) * 512],
                                start=True, stop=True,
                                perf_mode=PM.DoubleRow,
                            )
                        ys_col = yscs[dr][:, jt:jt + 1]
                        if SKIP_STREAM:
                            continue
                        if jt < ACT_TILES:
                            # ACT stream: copy+bias to bf16, DVE TT-min merge
                            gb = gb_p.tile([128, NPC], BF16, tag="gb")
                            nc.scalar.activation(
                                gb[:], g[:], AF.Identity, bias=ys_col,
                                scale=1.0)
                            if jt == 0:
                                runs[dr] = gb
                            else:
                                mg = mg_p.tile([128, NPC], BF16, tag="mg")
                                nc.vector.tensor_tensor(
                                    out=mg[:], in0=runs[dr][:], in1=gb[:],
                                    op=ALU.min)
                                runs[dr] = mg
                        else:
                            # DVE stt chain (absorbs the ACT merge as in1)
                            nrun = run_p.tile([128, NPC], BF16, tag="run")
                            if runs[dr] is None:
                                nc.vector.tensor_scalar(
                                    out=nrun[:], in0=g[:], scalar1=ys_col,
                                    scalar2=None, op0=ALU.add)
                            else:
                                nc.vector.scalar_tensor_tensor(
                                    out=nrun[:], in0=g[:], scalar=ys_col,
                                    in1=runs[dr][:], op0=ALU.add,
                                    op1=ALU.min)
                            runs[dr] = nrun
                state[("runs", c)] = runs

            def finish(c):
                runs = state.pop(("runs", c))
                for dr in range(2):
                    if SKIP_FINISH or SKIP_STREAM:
                        break
                    ft = psg_p.tile([128, JT, 128], BF16, tag="g")
                    for ib in range(JT):
                        nc.tensor.transpose(
                            ft[:, ib, :],
                            runs[dr][:, ib * 128:(ib + 1) * 128],
                            idb[:],
                        )
                    base = dr * 64 + c * 8
                    nc.vector.tensor_reduce(
                        out=pmin[:, base:base + JT], in_=ft[:],
                        axis=AX.X, op=ALU.min)

            for c in range(DEPTH):
                prep(c)
            for c in range(CPC):
                pairwise(c)
                if c > 0:
                    finish(c - 1)
                if c + DEPTH < CPC:
                    prep(c + DEPTH)
            finish(CPC - 1)

            # ---- finals ----
            if DEBUG:
                nc.sync.dma_start(dpmin_d[:], pmin[:])
            red = const.tile([128, 2 * CPC], FP32)
            nc.vector.tensor_reduce(
                out=red[:],
                in_=pmin[:].rearrange("p (g k) -> p g k", k=JT),
                axis=AX.X, op=ALU.add)
            psf = psx_p.tile([1, 2 * CPC], FP32, tag="xf")
            nc.tensor.matmul(psf[:], onesc[:, 0:1], red[:], start=True,
                             stop=True)
            outrow = const.tile([1, 2 * CPC], FP32)
            nc.vector.tensor_tensor(
                out=outrow[:], in0=psf[:], in1=finsc[:], op=ALU.mult)
            nc.sync.dma_start(out_d[:], outrow[:])

    nc.compile()
    return nc


def _get_nc():
    if "nc" not in _CACHE:
        _CACHE["nc"] = _build_bass()
    return _CACHE["nc"]


def _pow2_below(x):
    return 2.0 ** math.floor(math.log2(x))


def kernel(protos1, protos2, W, b, num_classes):
    import ml_dtypes
    from concourse.bass_utils import run_bass_kernel_spmd

    nc_classes = int(num_classes)
    assert nc_classes == C and protos1.shape == (P, D)

    protos1 = np.ascontiguousarray(protos1, dtype=np.float32)
    protos2 = np.ascontiguousarray(protos2, dtype=np.float32)
    W = np.asarray(W, dtype=np.float32)
    b = np.asarray(b, dtype=np.float32)

    # transform matrices (lhsT [d, d']) with the -2 scale folded in
    V = np.linalg.inv(W.T.astype(np.float64)).astype(np.float32)
    V2 = (-2.0 * V).astype(np.float32)
    Wt2 = (-2.0 * W.T).astype(np.float32)
    bias0 = (2.0 * (b.astype(np.float64) @ V.astype(np.float64))).astype(
        np.float32)                      # dir0: +2*(b@V)
    bias1 = (-2.0 * b).astype(np.float32)

    # fp8 scales (powers of two, bounded to e4m3 range 240)
    mx = max(np.abs(protos1).max(), np.abs(protos2).max())
    sx = _pow2_below(224.0 / mx)
    n1 = np.sqrt((protos1.astype(np.float64) ** 2).sum(1))
    n2b = np.sqrt(((protos2.astype(np.float64) - b) ** 2).sum(1))
    colV = np.sqrt((V.astype(np.float64) ** 2).sum(0)).max()
    colW = np.sqrt((W.T.astype(np.float64) ** 2).sum(0)).max()
    B0 = 2.0 * n2b.max() * colV
    B1 = 2.0 * (n1.max() * colW + np.abs(b).max())
    sy0 = _pow2_below(224.0 / B0)
    sy1 = _pow2_below(224.0 / B1)

    # d-major class-sliced tables: (C, NPC, D) -> (C, D, NPC) -> (C,2,128,NPC)
    def dmajor(p):
        pc = p.reshape(NPC, C, D).transpose(1, 2, 0)      # (C, D, NPC)
        return np.ascontiguousarray(pc).reshape(C, 2, 128, NPC)

    p1t = dmajor(protos1)
    p2t = dmajor(protos2)
    q1t = (p1t * np.float32(sx)).astype(ml_dtypes.float8_e4m3)
    q2t = (p2t * np.float32(sx)).astype(ml_dtypes.float8_e4m3)

    # host xs: mean_i |x_i|^2 per class from the quantized tables
    def xsm_of(q):
        f = q.astype(np.float32).astype(np.float64) / sx
        return (f ** 2).sum(axis=(1, 2)).mean(axis=1)     # (C,)

    xsm = np.stack([xsm_of(q1t), xsm_of(q2t)]).astype(np.float64)  # (2, C)

    mats = np.stack([
        np.stack([V2[0:128, :], V2[128:256, :]]),
        np.stack([Wt2[0:128, :], Wt2[128:256, :]]),
    ]).astype(np.float32)                                 # [2, 2, 128, 256]
    ssq0 = math.sqrt(sx * sy0) / 2.0
    ssq1 = math.sqrt(sx * sy1) / 2.0
    biases = np.stack([
        np.concatenate([(bias0 * sy0).reshape(2, 128).T,
                        np.full((128, 1), sy0, np.float32),
                        (bias0 * ssq0).reshape(2, 128).T,
                        np.full((128, 1), ssq0, np.float32)], axis=1),
        np.concatenate([(bias1 * sy1).reshape(2, 128).T,
                        np.full((128, 1), sy1, np.float32),
                        (bias1 * ssq1).reshape(2, 128).T,
                        np.full((128, 1), ssq1, np.float32)], axis=1),
    ]).astype(np.float32)                                 # [2, 128, 6]
    onesc = np.ones((128, 3), dtype=np.float32)
    idb = np.eye(128, dtype=np.float32).astype(ml_dtypes.bfloat16)
    finsc = np.concatenate([
        np.full(CPC, 1.0 / (NPC * sx * sy0), np.float64),
        np.full(CPC, 1.0 / (NPC * sx * sy1), np.float64),
    ]).astype(np.float32).reshape(1, 2 * CPC)

    in_maps = []
    for core in range(N_CORES):
        sl = slice(core * CPC, (core + 1) * CPC)
        in_maps.append({
            "p1t": np.ascontiguousarray(p1t[sl]),
            "p2t": np.ascontiguousarray(p2t[sl]),
            "q1t": np.ascontiguousarray(q1t[sl]),
            "q2t": np.ascontiguousarray(q2t[sl]),
            "mats": mats,
            "biases": biases,
            "onesc": onesc,
            "idb": idb,
            "finsc": finsc,
        })

    nc = _get_nc()
    res = run_bass_kernel_spmd(nc, in_maps, core_ids=list(range(N_CORES)))
    _CACHE["last_result"] = res

    out = np.zeros((2, C), dtype=np.float64)
    for core in range(N_CORES):
        row = res.results[core]["out"].reshape(2, CPC).astype(np.float64)
        sl = slice(core * CPC, (core + 1) * CPC)
        out[0, sl] = row[0] + xsm[0, sl]
        out[1, sl] = row[1] + xsm[1, sl]
    return out.astype(np.float32)


# revision 32
# speedup vs baseline: 1.2708x; 1.0230x over previous
"""Trainium2 Bass kernel for nn_ProtoCycleModel (retrieval_knn), v2.

Problem: P=65536 prototypes, C=64 classes, D=256.
Per class c (rows c::64 of each table, n=1024):
    loss_src[c] = mean_i min_j ||p1_c[i] - inv(W.T)@(p2_c[j]-b)||^2
    loss_tgt[c] = mean_i min_j ||p2_c[i] - (W.T@p1_c[j]+b)||^2
Output: (2, 64) fp32.  Sharding: 8 classes per core.

Design ("flipped layout"):
  - Host sends tables d-major (C, 2, 128, NPC) as fp32(r) AND fp8e4
    (scaled by power-of-2 sx); host also precomputes mean|x|^2 per class
    (added to the device result at the end, like the inv(W) prep).
  - Transform y' = M@x + b on PE in fp32r -> yt8 (fp8, scale sy_dr) via ACT.
  - sq = Square(ssq*(transform+bias)) on ACT from the pre-quantization
    psum (critical for accuracy); ys columns [128, 8] via per-j-tile
    N=1 matmuls with sq as stationary and a ones column as moving.
  - Pairwise G'[j%128, i] = sum_d qx[d,i] * yt8[d,j]: ONE fp8 DoubleRow
    matmul per 128-j tile (K=256 in one pass, 0.5 cycles/row).
  - j sits on PSUM partitions, so +|y'|^2 is a per-partition scalar:
    DVE scalar_tensor_tensor fuses (G + ys) and running min across
    j-tiles in the single required PSUM pass; ACT_TILES j-tiles per
    class-dir instead go through ACT activation(bias=ys_col) -> bf16
    copies merged by DVE tensor_tensor min at the 2x bf16 rate
    (GPSIMD has no PSUM port and no min/max ops, so Pool cannot help).
  - Finish per class-dir (deferred one class for overlap): 8 PE
    transposes of the [128,1024] bf16 running min -> psum [128, 8, 128],
    one DVE min-reduce -> pmin columns; final: add-reduce, ones-matmul
    cross-partition sum, per-dir descale, DMA out; host adds mean|x|^2.
  Timeline-sim: 197561 ns vs 251064 ns baseline; rel err 5.6e-4.
"""

import math
import os

import numpy as np

P, C, D = 65536, 64, 256
N_CORES = 8
CPC = C // N_CORES          # classes per core = 8
NPC = P // C                # prototypes per class = 1024
JT = NPC // 128             # j-tiles per class-dir = 8

# ACT-streamed j-tiles per class-dir (rest go through the DVE stt chain)
ACT_TILES = int(os.environ.get("K_ACT_TILES", "5"))
SKIP_FINISH = os.environ.get("K_SKIP_FINISH", "0") == "1"
SKIP_STREAM = os.environ.get("K_SKIP_STREAM", "0") == "1"
SKIP_YS = os.environ.get("K_SKIP_YS", "0") == "1"
DEPTH = int(os.environ.get("K_DEPTH", "2"))
BUFS = DEPTH + 1
GBUFS = int(os.environ.get("K_GBUFS", "2"))
XBUFS = int(os.environ.get("K_XBUFS", "2"))
UPLACE = int(os.environ.get("K_UPLACE", "0"))  # 1=interleave, 0=after loop
FPLACE = int(os.environ.get("K_FPLACE", "2"))

_CACHE = {}


def _build_bass():
    from concourse import bacc
    import concourse.tile as tile
    from concourse import mybir

    FP32 = mybir.dt.float32
    FP32R = mybir.dt.float32r
    BF16 = mybir.dt.bfloat16
    FP8 = mybir.dt.float8e4
    AF = mybir.ActivationFunctionType
    ALU = mybir.AluOpType
    AX = mybir.AxisListType
    PM = mybir.MatmulPerfMode

    nc = bacc.Bacc(None, target_bir_lowering=False)

    p1t_d = nc.dram_tensor("p1t", [CPC, 2, 128, NPC], FP32R, kind="ExternalInput")
    p2t_d = nc.dram_tensor("p2t", [CPC, 2, 128, NPC], FP32R, kind="ExternalInput")
    q1t_d = nc.dram_tensor("q1t", [CPC, 2, 128, NPC], FP8, kind="ExternalInput")
    q2t_d = nc.dram_tensor("q2t", [CPC, 2, 128, NPC], FP8, kind="ExternalInput")
    # mats[dir][dc]: [128, 256] fp32r, lhsT [d, d'] with -2 folded in
    mats_d = nc.dram_tensor("mats", [2, 2, 128, D], FP32R, kind="ExternalInput")
    # biases[dir][dcp] per-partition: sy_dr * bias_raw_dr
    bias_d = nc.dram_tensor("biases", [2, 128, 6], FP32, kind="ExternalInput")
    # consts cols: 0 = ones 1.0, 1..2 = sx/(4*sy_dr)
    ones_d = nc.dram_tensor("onesc", [128, 3], FP32, kind="ExternalInput")
    idb_d = nc.dram_tensor("idb", [128, 128], BF16, kind="ExternalInput")
    finsc_d = nc.dram_tensor("finsc", [1, 2 * CPC], FP32, kind="ExternalInput")
    out_d = nc.dram_tensor("out", [1, 2 * CPC], FP32, kind="ExternalOutput")
    DEBUG = os.environ.get("K_DEBUG", "0") == "1"
    if DEBUG:
        dpmin_d = nc.dram_tensor("dpmin", [128, 2 * CPC * JT], FP32,
                                 kind="ExternalOutput")
        dysc_d = nc.dram_tensor("dysc", [2, 128, JT], FP32,
                                kind="ExternalOutput")
        dyt8_d = nc.dram_tensor("dyt8", [128, 2, NPC], FP32,
                                kind="ExternalOutput")



    with tile.TileContext(nc) as tc:
        with (
            tc.tile_pool(name="const", bufs=1) as const,
            tc.tile_pool(name="xt", bufs=BUFS) as xt_p,
            tc.tile_pool(name="qx", bufs=BUFS) as qx_p,
            tc.tile_pool(name="yt", bufs=BUFS) as yt_p,
            tc.tile_pool(name="sq", bufs=BUFS) as sq_p,
            tc.tile_pool(name="ysc", bufs=BUFS) as ysc_p,
            tc.tile_pool(name="run", bufs=10) as run_p,
            tc.tile_pool(name="gb", bufs=4) as gb_p,
            tc.tile_pool(name="mg", bufs=8) as mg_p,
            tc.tile_pool(name="psg", bufs=GBUFS, space="PSUM") as psg_p,
            tc.tile_pool(name="psx", bufs=XBUFS, space="PSUM") as psx_p,
        ):
            # ---- constants ----
            mats = const.tile([128, 2, 2, D], FP32R)
            nc.sync.dma_start(mats[:], mats_d[:].rearrange("a b p d -> p a b d"))
            biases = const.tile([128, 2, 6], FP32)
            nc.sync.dma_start(biases[:], bias_d[:].rearrange("a p c -> p a c"))
            onesc = const.tile([128, 3], FP32)
            nc.sync.dma_start(onesc[:], ones_d[:])
            idb = const.tile([128, 128], BF16)
            nc.sync.dma_start(idb[:], idb_d[:])
            finsc = const.tile([1, 2 * CPC], FP32)
            nc.sync.dma_start(finsc[:], finsc_d[:])

            pmin = const.tile([128, 2 * CPC * JT], FP32)  # col = dr*64+c*8+ib
            if SKIP_FINISH or SKIP_STREAM:
                nc.vector.memset(pmin[:], 0.0)

            state = {}

            def dma_in(c):
                xts, qxs = [], []
                for t, (src_d, qsrc_d) in enumerate(
                    ((p1t_d, q1t_d), (p2t_d, q2t_d))
                ):
                    xt = xt_p.tile([128, 2, NPC], FP32R, tag=f"xt{t}")
                    nc.sync.dma_start(
                        xt[:], src_d[c].rearrange("a p j -> p a j"))
                    qx = qx_p.tile([128, 2, NPC], FP8, tag=f"qx{t}")
                    nc.sync.dma_start(
                        qx[:], qsrc_d[c].rearrange("a p j -> p a j"))
                    xts.append(xt)
                    qxs.append(qx)
                state[("in", c)] = (xts, qxs)

            def transform_units(c):
                """8 closures, each: 2 PE matmuls (one pstf half) + 2 ACT."""
                xts, qxs = state[("in", c)]
                yt8s, sqs = [], []
                for dr in range(2):
                    yt8 = yt_p.tile([128, 2, NPC], FP8, tag=f"yt{dr}",
                                    name=f"yt8_{c}_{dr}")
                    sq = sq_p.tile([128, 2, NPC], FP32, tag=f"sq{dr}",
                                   name=f"sq_{c}_{dr}")
                    yt8s.append(yt8)
                    sqs.append(sq)
                units = []
                for dr in range(2):
                    for dcp in range(2):
                        def unit(dr=dr, dcp=dcp):
                            ysrc = xts[1 - dr]
                            pstf = psx_p.tile([128, NPC], FP32, tag="xf")
                            for dc in range(2):
                                for ih in range(2):
                                    nc.tensor.matmul(
                                        pstf[:, ih * 512:(ih + 1) * 512],
                                        mats[:, dr, dc,
                                             dcp * 128:(dcp + 1) * 128],
                                        ysrc[:, dc, ih * 512:(ih + 1) * 512],
                                        start=(dc == 0), stop=(dc == 1),
                                    )
                            nc.scalar.activation(
                                sqs[dr][:, dcp, :], pstf[:], AF.Square,
                                bias=biases[:, dr, 3 + dcp:4 + dcp],
                                scale=biases[:, dr, 5:6])
                            nc.scalar.activation(
                                yt8s[dr][:, dcp, :], pstf[:], AF.Identity,
                                bias=biases[:, dr, dcp:dcp + 1],
                                scale=biases[:, dr, 2:3])
                        units.append(unit)
                state[("yt", c)] = (yt8s, sqs)
                return units

            def ys_finalize(c):
                yt8s, sqs = state[("yt", c)]
                yscs = []
                for dr in range(2):
                    ysp2 = psx_p.tile([128, JT], FP32, tag="xf")
                    for jt in range(JT):
                        for dcp in range(2):
                            nc.tensor.matmul(
                                ysp2[:, jt:jt + 1],
                                sqs[dr][:, dcp, jt * 128:(jt + 1) * 128],
                                onesc[:, 0:1],
                                start=(dcp == 0), stop=(dcp == 1),
                            )
                    ysc = ysc_p.tile([128, JT], FP32, tag=f"ys{dr}")
                    nc.vector.tensor_copy(ysc[:], ysp2[:])
                    yscs.append(ysc)
                _, qxs = state.pop(("in", c))
                state[c] = (qxs, yt8s, yscs)

            def finish(c):
                runs = state.pop(("runs", c))
                for dr in range(2):
                    if SKIP_FINISH or SKIP_STREAM:
                        break
                    ft = psg_p.tile([128, JT, 128], BF16, tag="g")
                    for ib in range(JT):
                        nc.tensor.transpose(
                            ft[:, ib, :],
                            runs[dr][:, ib * 128:(ib + 1) * 128],
                            idb[:],
                        )
                    base = dr * 64 + c * 8
                    nc.vector.tensor_reduce(
                        out=pmin[:, base:base + JT], in_=ft[:],
                        axis=AX.X, op=ALU.min)

            def pairwise(c, units):
                qxs, yt8s, yscs = state.pop(c)
                runs = [None, None]
                for jt in range(JT):
                    for dr in range(2):
                        g = psg_p.tile([128, NPC], FP32, tag="g")
                        for jh in range(2):
                            nc.tensor.matmul(
                                g[:, jh * 512:(jh + 1) * 512],
                                yt8s[dr][:, :, jt * 128:(jt + 1) * 128],
                                qxs[dr][:, :, jh * 512:(jh + 1) * 512],
                                start=True, stop=True,
                                perf_mode=PM.DoubleRow,
                            )
                        ys_col = yscs[dr][:, jt:jt + 1]
                        if SKIP_STREAM:
                            continue
                        if jt < ACT_TILES:
                            gb = gb_p.tile([128, NPC], BF16, tag="gb")
                            nc.scalar.activation(
                                gb[:], g[:], AF.Identity, bias=ys_col,
                                scale=1.0)
                            if jt == 0:
                                runs[dr] = gb
                            else:
                                mg = mg_p.tile([128, NPC], BF16, tag="mg")
                                nc.vector.tensor_tensor(
                                    out=mg[:], in0=runs[dr][:], in1=gb[:],
                                    op=ALU.min)
                                runs[dr] = mg
                        else:
                            nrun = run_p.tile([128, NPC], BF16, tag="run")
                            if runs[dr] is None:
                                nc.vector.tensor_scalar(
                                    out=nrun[:], in0=g[:], scalar1=ys_col,
                                    scalar2=None, op0=ALU.add)
                            else:
                                nc.vector.scalar_tensor_tensor(
                                    out=nrun[:], in0=g[:], scalar=ys_col,
                                    in1=runs[dr][:], op0=ALU.add,
                                    op1=ALU.min)
                            runs[dr] = nrun
                    if UPLACE and jt % 2 == 0 and jt // 2 < len(units):
                        units[jt // 2]()
                    if jt == FPLACE and ("runs", c - 1) in state:
                        finish(c - 1)
                if not UPLACE:
                    for u in units:
                        u()
                state[("runs", c)] = runs

            dma_in(0)
            dma_in(1)
            for u in transform_units(0):
                u()
            ys_finalize(0)
            for c in range(CPC):
                if c + 2 < CPC:
                    dma_in(c + 2)
                units = transform_units(c + 1) if c + 1 < CPC else []
                pairwise(c, units)
                if c + 1 < CPC:
                    ys_finalize(c + 1)
            finish(CPC - 1)

            # ---- finals ----
            if DEBUG:
                nc.sync.dma_start(dpmin_d[:], pmin[:])
            red = const.tile([128, 2 * CPC], FP32)
            nc.vector.tensor_reduce(
                out=red[:],
                in_=pmin[:].rearrange("p (g k) -> p g k", k=JT),
                axis=AX.X, op=ALU.add)
            psf = psx_p.tile([1, 2 * CPC], FP32, tag="xf")
            nc.tensor.matmul(psf[:], onesc[:, 0:1], red[:], start=True,
                             stop=True)
            outrow = const.tile([1, 2 * CPC], FP32)
            nc.vector.tensor_tensor(
                out=outrow[:], in0=psf[:], in1=finsc[:], op=ALU.mult)
            nc.sync.dma_start(out_d[:], outrow[:])

    nc.compile()
    return nc


def _get_nc():
    if "nc" not in _CACHE:
        _CACHE["nc"] = _build_bass()
    return _CACHE["nc"]


def _pow2_below(x):
    return 2.0 ** math.floor(math.log2(x))


def kernel(protos1, protos2, W, b, num_classes):
    import ml_dtypes
    from concourse.bass_utils import run_bass_kernel_spmd

    nc_classes = int(num_classes)
    assert nc_classes == C and protos1.shape == (P, D)

    protos1 = np.ascontiguousarray(protos1, dtype=np.float32)
    protos2 = np.ascontiguousarray(protos2, dtype=np.float32)
    W = np.asarray(W, dtype=np.float32)
    b = np.asarray(b, dtype=np.float32)

    # transform matrices (lhsT [d, d']) with the -2 scale folded in
    V = np.linalg.inv(W.T.astype(np.float64)).astype(np.float32)
    V2 = (-2.0 * V).astype(np.float32)
    Wt2 = (-2.0 * W.T).astype(np.float32)
    bias0 = (2.0 * (b.astype(np.float64) @ V.astype(np.float64))).astype(
        np.float32)                      # dir0: +2*(b@V)
    bias1 = (-2.0 * b).astype(np.float32)

    # fp8 scales (powers of two, bounded to e4m3 range 240)
    mx = max(np.abs(protos1).max(), np.abs(protos2).max())
    sx = _pow2_below(224.0 / mx)
    n1 = np.sqrt((protos1.astype(np.float64) ** 2).sum(1))
    n2b = np.sqrt(((protos2.astype(np.float64) - b) ** 2).sum(1))
    colV = np.sqrt((V.astype(np.float64) ** 2).sum(0)).max()
    colW = np.sqrt((W.T.astype(np.float64) ** 2).sum(0)).max()
    B0 = 2.0 * n2b.max() * colV
    B1 = 2.0 * (n1.max() * colW + np.abs(b).max())
    sy0 = _pow2_below(224.0 / B0)
    sy1 = _pow2_below(224.0 / B1)

    # d-major class-sliced tables: (C, NPC, D) -> (C, D, NPC) -> (C,2,128,NPC)
    def dmajor(p):
        pc = p.reshape(NPC, C, D).transpose(1, 2, 0)      # (C, D, NPC)
        return np.ascontiguousarray(pc).reshape(C, 2, 128, NPC)

    p1t = dmajor(protos1)
    p2t = dmajor(protos2)
    q1t = (p1t * np.float32(sx)).astype(ml_dtypes.float8_e4m3)
    q2t = (p2t * np.float32(sx)).astype(ml_dtypes.float8_e4m3)

    # host xs: mean_i |x_i|^2 per class from the quantized tables
    def xsm_of(q):
        f = q.astype(np.float32).astype(np.float64) / sx
        return (f ** 2).sum(axis=(1, 2)).mean(axis=1)     # (C,)

    xsm = np.stack([xsm_of(q1t), xsm_of(q2t)]).astype(np.float64)  # (2, C)

    mats = np.stack([
        np.stack([V2[0:128, :], V2[128:256, :]]),
        np.stack([Wt2[0:128, :], Wt2[128:256, :]]),
    ]).astype(np.float32)                                 # [2, 2, 128, 256]
    ssq0 = math.sqrt(sx * sy0) / 2.0
    ssq1 = math.sqrt(sx * sy1) / 2.0
    biases = np.stack([
        np.concatenate([(bias0 * sy0).reshape(2, 128).T,
                        np.full((128, 1), sy0, np.float32),
                        (bias0 * ssq0).reshape(2, 128).T,
                        np.full((128, 1), ssq0, np.float32)], axis=1),
        np.concatenate([(bias1 * sy1).reshape(2, 128).T,
                        np.full((128, 1), sy1, np.float32),
                        (bias1 * ssq1).reshape(2, 128).T,
                        np.full((128, 1), ssq1, np.float32)], axis=1),
    ]).astype(np.float32)                                 # [2, 128, 6]
    onesc = np.ones((128, 3), dtype=np.float32)
    idb = np.eye(128, dtype=np.float32).astype(ml_dtypes.bfloat16)
    finsc = np.concatenate([
        np.full(CPC, 1.0 / (NPC * sx * sy0), np.float64),
        np.full(CPC, 1.0 / (NPC * sx * sy1), np.float64),
    ]).astype(np.float32).reshape(1, 2 * CPC)

    in_maps = []
    for core in range(N_CORES):
        sl = slice(core * CPC, (core + 1) * CPC)
        in_maps.append({
            "p1t": np.ascontiguousarray(p1t[sl]),
            "p2t": np.ascontiguousarray(p2t[sl]),
            "q1t": np.ascontiguousarray(q1t[sl]),
            "q2t": np.ascontiguousarray(q2t[sl]),
            "mats": mats,
            "biases": biases,
            "onesc": onesc,
            "idb": idb,
            "finsc": finsc,
        })

    nc = _get_nc()
    res = run_bass_kernel_spmd(nc, in_maps, core_ids=list(range(N_CORES)))
    _CACHE["last_result"] = res

    out = np.zeros((2, C), dtype=np.float64)
    for core in range(N_CORES):
        row = res.results[core]["out"].reshape(2, CPC).astype(np.float64)
        sl = slice(core * CPC, (core + 1) * CPC)
        out[0, sl] = row[0] + xsm[0, sl]
        out[1, sl] = row[1] + xsm[1, sl]
    return out.astype(np.float32)


# revision 34
# speedup vs baseline: 1.3744x; 1.0815x over previous
"""Trainium2 Bass kernel for nn_ProtoCycleModel (retrieval_knn), v2.

Problem: P=65536 prototypes, C=64 classes, D=256.
Per class c (rows c::64 of each table, n=1024):
    loss_src[c] = mean_i min_j ||p1_c[i] - inv(W.T)@(p2_c[j]-b)||^2
    loss_tgt[c] = mean_i min_j ||p2_c[i] - (W.T@p1_c[j]+b)||^2
Output: (2, 64) fp32.  Sharding: 8 classes per core.

Design ("flipped layout"):
  - Host sends tables d-major (C, 2, 128, NPC) as fp32(r) AND fp8e4
    (scaled by power-of-2 sx); host also precomputes mean|x|^2 per class
    (added to the device result at the end, like the inv(W) prep).
  - Transform y' = M@x + b on PE in fp32r -> yt8 (fp8, scale sy_dr) via ACT.
  - sq = Square(ssq*(transform+bias)) on ACT from the pre-quantization
    psum (critical for accuracy); ys columns [128, 8] via per-j-tile
    N=1 matmuls with sq as stationary and a ones column as moving.
  - Pairwise G'[j%128, i] = sum_d qx[d,i] * yt8[d,j]: ONE fp8 DoubleRow
    matmul per 128-j tile (K=256 in one pass, 0.5 cycles/row).
  - j sits on PSUM partitions, so +|y'|^2 is a per-partition scalar:
    DVE scalar_tensor_tensor fuses (G + ys) and running min across
    j-tiles in the single required PSUM pass; ACT_TILES j-tiles per
    class-dir instead go through ACT activation(bias=ys_col) -> bf16
    copies merged by DVE tensor_tensor min at the 2x bf16 rate
    (GPSIMD has no PSUM port and no min/max ops, so Pool cannot help).
  - Finish per class-dir (deferred one class for overlap): 8 PE
    transposes of the [128,1024] bf16 running min -> psum [128, 8, 128],
    one DVE min-reduce -> pmin columns; final: add-reduce, ones-matmul
    cross-partition sum, per-dir descale, DMA out; host adds mean|x|^2.
  Timeline-sim: 182677 ns vs 251064 ns baseline; rel err 5.6e-4.
"""

import math
import os

import numpy as np

P, C, D = 65536, 64, 256
N_CORES = 8
CPC = C // N_CORES          # classes per core = 8
NPC = P // C                # prototypes per class = 1024
JT = NPC // 128             # j-tiles per class-dir = 8

# ACT-streamed j-tiles per class-dir (rest go through the DVE stt chain)
ACT_TILES = int(os.environ.get("K_ACT_TILES", "4"))
# alternate ACT/DVE tiles so the merge chain pipelines tile-by-tile
_PATTERNS = {
    0: [], 1: [0], 2: [0, 4], 3: [0, 3, 6], 4: [0, 2, 4, 6],
    5: [0, 2, 4, 6, 7], 6: [0, 1, 2, 4, 5, 6], 7: [0, 1, 2, 3, 4, 5, 6],
    8: list(range(8)),
}
ACT_SET = set(_PATTERNS[ACT_TILES])
SKIP_FINISH = os.environ.get("K_SKIP_FINISH", "0") == "1"
SKIP_STREAM = os.environ.get("K_SKIP_STREAM", "0") == "1"
SKIP_YS = os.environ.get("K_SKIP_YS", "0") == "1"
DEPTH = int(os.environ.get("K_DEPTH", "2"))
BUFS = DEPTH + 1
GBUFS = int(os.environ.get("K_GBUFS", "2"))
XBUFS = int(os.environ.get("K_XBUFS", "2"))
UPLACE = int(os.environ.get("K_UPLACE", "0"))  # 1=interleave, 0=after loop
FPLACE = int(os.environ.get("K_FPLACE", "6"))

_CACHE = {}


def _build_bass():
    from concourse import bacc
    import concourse.tile as tile
    from concourse import mybir

    FP32 = mybir.dt.float32
    FP32R = mybir.dt.float32r
    BF16 = mybir.dt.bfloat16
    FP8 = mybir.dt.float8e4
    AF = mybir.ActivationFunctionType
    ALU = mybir.AluOpType
    AX = mybir.AxisListType
    PM = mybir.MatmulPerfMode

    nc = bacc.Bacc(None, target_bir_lowering=False)

    p1t_d = nc.dram_tensor("p1t", [CPC, 2, 128, NPC], FP32R, kind="ExternalInput")
    p2t_d = nc.dram_tensor("p2t", [CPC, 2, 128, NPC], FP32R, kind="ExternalInput")
    q1t_d = nc.dram_tensor("q1t", [CPC, 2, 128, NPC], FP8, kind="ExternalInput")
    q2t_d = nc.dram_tensor("q2t", [CPC, 2, 128, NPC], FP8, kind="ExternalInput")
    # mats[dir][dc]: [128, 256] fp32r, lhsT [d, d'] with -2 folded in
    mats_d = nc.dram_tensor("mats", [2, 2, 128, D], FP32R, kind="ExternalInput")
    # biases[dir][dcp] per-partition: sy_dr * bias_raw_dr
    bias_d = nc.dram_tensor("biases", [2, 128, 6], FP32, kind="ExternalInput")
    # consts cols: 0 = ones 1.0, 1..2 = sx/(4*sy_dr)
    ones_d = nc.dram_tensor("onesc", [128, 3], FP32, kind="ExternalInput")
    idb_d = nc.dram_tensor("idb", [128, 128], BF16, kind="ExternalInput")
    finsc_d = nc.dram_tensor("finsc", [1, 2 * CPC], FP32, kind="ExternalInput")
    out_d = nc.dram_tensor("out", [1, 2 * CPC], FP32, kind="ExternalOutput")
    DEBUG = os.environ.get("K_DEBUG", "0") == "1"
    if DEBUG:
        dpmin_d = nc.dram_tensor("dpmin", [128, 2 * CPC * JT], FP32,
                                 kind="ExternalOutput")
        dysc_d = nc.dram_tensor("dysc", [2, 128, JT], FP32,
                                kind="ExternalOutput")
        dyt8_d = nc.dram_tensor("dyt8", [128, 2, NPC], FP32,
                                kind="ExternalOutput")



    with tile.TileContext(nc) as tc:
        with (
            tc.tile_pool(name="const", bufs=1) as const,
            tc.tile_pool(name="xt", bufs=BUFS) as xt_p,
            tc.tile_pool(name="qx", bufs=BUFS) as qx_p,
            tc.tile_pool(name="yt", bufs=BUFS) as yt_p,
            tc.tile_pool(name="sq", bufs=BUFS) as sq_p,
            tc.tile_pool(name="ysc", bufs=BUFS) as ysc_p,
            tc.tile_pool(name="run", bufs=10) as run_p,
            tc.tile_pool(name="gb", bufs=4) as gb_p,
            tc.tile_pool(name="mg", bufs=8) as mg_p,
            tc.tile_pool(name="psg", bufs=GBUFS, space="PSUM") as psg_p,
            tc.tile_pool(name="psx", bufs=XBUFS, space="PSUM") as psx_p,
        ):
            # ---- constants ----
            mats = const.tile([128, 2, 2, D], FP32R)
            nc.sync.dma_start(mats[:], mats_d[:].rearrange("a b p d -> p a b d"))
            biases = const.tile([128, 2, 6], FP32)
            nc.sync.dma_start(biases[:], bias_d[:].rearrange("a p c -> p a c"))
            onesc = const.tile([128, 3], FP32)
            nc.sync.dma_start(onesc[:], ones_d[:])
            idb = const.tile([128, 128], BF16)
            nc.sync.dma_start(idb[:], idb_d[:])
            finsc = const.tile([1, 2 * CPC], FP32)
            nc.sync.dma_start(finsc[:], finsc_d[:])

            pmin = const.tile([128, 2 * CPC * JT], FP32)  # col = dr*64+c*8+ib
            if SKIP_FINISH or SKIP_STREAM:
                nc.vector.memset(pmin[:], 0.0)

            state = {}

            def dma_in(c):
                xts, qxs = [], []
                for t, (src_d, qsrc_d) in enumerate(
                    ((p1t_d, q1t_d), (p2t_d, q2t_d))
                ):
                    xt = xt_p.tile([128, 2, NPC], FP32R, tag=f"xt{t}")
                    nc.sync.dma_start(
                        xt[:], src_d[c].rearrange("a p j -> p a j"))
                    qx = qx_p.tile([128, 2, NPC], FP8, tag=f"qx{t}")
                    nc.sync.dma_start(
                        qx[:], qsrc_d[c].rearrange("a p j -> p a j"))
                    xts.append(xt)
                    qxs.append(qx)
                state[("in", c)] = (xts, qxs)

            def transform_units(c):
                """8 closures, each: 2 PE matmuls (one pstf half) + 2 ACT."""
                xts, qxs = state[("in", c)]
                yt8s, sqs = [], []
                for dr in range(2):
                    yt8 = yt_p.tile([128, 2, NPC], FP8, tag=f"yt{dr}",
                                    name=f"yt8_{c}_{dr}")
                    sq = sq_p.tile([128, 2, NPC], FP32, tag=f"sq{dr}",
                                   name=f"sq_{c}_{dr}")
                    yt8s.append(yt8)
                    sqs.append(sq)
                units = []
                for dr in range(2):
                    for dcp in range(2):
                        def unit(dr=dr, dcp=dcp):
                            ysrc = xts[1 - dr]
                            pstf = psx_p.tile([128, NPC], FP32, tag="xf")
                            for dc in range(2):
                                for ih in range(2):
                                    nc.tensor.matmul(
                                        pstf[:, ih * 512:(ih + 1) * 512],
                                        mats[:, dr, dc,
                                             dcp * 128:(dcp + 1) * 128],
                                        ysrc[:, dc, ih * 512:(ih + 1) * 512],
                                        start=(dc == 0), stop=(dc == 1),
                                    )
                            nc.scalar.activation(
                                sqs[dr][:, dcp, :], pstf[:], AF.Square,
                                bias=biases[:, dr, 3 + dcp:4 + dcp],
                                scale=biases[:, dr, 5:6])
                            nc.scalar.activation(
                                yt8s[dr][:, dcp, :], pstf[:], AF.Identity,
                                bias=biases[:, dr, dcp:dcp + 1],
                                scale=biases[:, dr, 2:3])
                        units.append(unit)
                state[("yt", c)] = (yt8s, sqs)
                return units

            def ys_finalize(c):
                yt8s, sqs = state[("yt", c)]
                yscs = []
                for dr in range(2):
                    ysp2 = psx_p.tile([128, JT], FP32, tag="xf")
                    for jt in range(JT):
                        for dcp in range(2):
                            nc.tensor.matmul(
                                ysp2[:, jt:jt + 1],
                                sqs[dr][:, dcp, jt * 128:(jt + 1) * 128],
                                onesc[:, 0:1],
                                start=(dcp == 0), stop=(dcp == 1),
                            )
                    ysc = ysc_p.tile([128, JT], FP32, tag=f"ys{dr}")
                    nc.vector.tensor_copy(ysc[:], ysp2[:])
                    yscs.append(ysc)
                _, qxs = state.pop(("in", c))
                state[c] = (qxs, yt8s, yscs)

            def finish(c):
                runs = state.pop(("runs", c))
                for dr in range(2):
                    if SKIP_FINISH or SKIP_STREAM:
                        break
                    ft = psg_p.tile([128, JT, 128], BF16, tag="g")
                    for ib in range(JT):
                        nc.tensor.transpose(
                            ft[:, ib, :],
                            runs[dr][:, ib * 128:(ib + 1) * 128],
                            idb[:],
                        )
                    base = dr * 64 + c * 8
                    nc.vector.tensor_reduce(
                        out=pmin[:, base:base + JT], in_=ft[:],
                        axis=AX.X, op=ALU.min)

            def pairwise(c, units):
                qxs, yt8s, yscs = state.pop(c)
                runs = [None, None]
                for jt in range(JT):
                    for dr in range(2):
                        g = psg_p.tile([128, NPC], FP32, tag="g")
                        for jh in range(2):
                            nc.tensor.matmul(
                                g[:, jh * 512:(jh + 1) * 512],
                                yt8s[dr][:, :, jt * 128:(jt + 1) * 128],
                                qxs[dr][:, :, jh * 512:(jh + 1) * 512],
                                start=True, stop=True,
                                perf_mode=PM.DoubleRow,
                            )
                        ys_col = yscs[dr][:, jt:jt + 1]
                        if SKIP_STREAM:
                            continue
                        if jt in ACT_SET:
                            # ACT reads PSUM; DVE merges into the chain
                            gb = gb_p.tile([128, NPC], BF16, tag="gb")
                            nc.scalar.activation(
                                gb[:], g[:], AF.Identity, bias=ys_col,
                                scale=1.0)
                            if runs[dr] is None:
                                runs[dr] = gb
                            else:
                                mg = mg_p.tile([128, NPC], BF16, tag="mg")
                                nc.vector.tensor_tensor(
                                    out=mg[:], in0=runs[dr][:], in1=gb[:],
                                    op=ALU.min)
                                runs[dr] = mg
                        else:
                            nrun = run_p.tile([128, NPC], BF16, tag="run")
                            if runs[dr] is None:
                                nc.vector.tensor_scalar(
                                    out=nrun[:], in0=g[:], scalar1=ys_col,
                                    scalar2=None, op0=ALU.add)
                            else:
                                nc.vector.scalar_tensor_tensor(
                                    out=nrun[:], in0=g[:], scalar=ys_col,
                                    in1=runs[dr][:], op0=ALU.add,
                                    op1=ALU.min)
                            runs[dr] = nrun
                    if UPLACE and jt % 2 == 0 and jt // 2 < len(units):
                        units[jt // 2]()
                    if jt == FPLACE and ("runs", c - 1) in state:
                        finish(c - 1)
                if not UPLACE:
                    for u in units:
                        u()
                state[("runs", c)] = runs

            dma_in(0)
            dma_in(1)
            for u in transform_units(0):
                u()
            ys_finalize(0)
            for c in range(CPC):
                if c + 2 < CPC:
                    dma_in(c + 2)
                units = transform_units(c + 1) if c + 1 < CPC else []
                pairwise(c, units)
                if c + 1 < CPC:
                    ys_finalize(c + 1)
            finish(CPC - 1)

            # ---- finals ----
            if DEBUG:
                nc.sync.dma_start(dpmin_d[:], pmin[:])
            red = const.tile([128, 2 * CPC], FP32)
            nc.vector.tensor_reduce(
                out=red[:],
                in_=pmin[:].rearrange("p (g k) -> p g k", k=JT),
                axis=AX.X, op=ALU.add)
            psf = psx_p.tile([1, 2 * CPC], FP32, tag="xf")
            nc.tensor.matmul(psf[:], onesc[:, 0:1], red[:], start=True,
                             stop=True)
            outrow = const.tile([1, 2 * CPC], FP32)
            nc.vector.tensor_tensor(
                out=outrow[:], in0=psf[:], in1=finsc[:], op=ALU.mult)
            nc.sync.dma_start(out_d[:], outrow[:])

    nc.compile()
    return nc


def _get_nc():
    if "nc" not in _CACHE:
        _CACHE["nc"] = _build_bass()
    return _CACHE["nc"]


def _pow2_below(x):
    return 2.0 ** math.floor(math.log2(x))


def kernel(protos1, protos2, W, b, num_classes):
    import ml_dtypes
    from concourse.bass_utils import run_bass_kernel_spmd

    nc_classes = int(num_classes)
    assert nc_classes == C and protos1.shape == (P, D)

    protos1 = np.ascontiguousarray(protos1, dtype=np.float32)
    protos2 = np.ascontiguousarray(protos2, dtype=np.float32)
    W = np.asarray(W, dtype=np.float32)
    b = np.asarray(b, dtype=np.float32)

    # transform matrices (lhsT [d, d']) with the -2 scale folded in
    V = np.linalg.inv(W.T.astype(np.float64)).astype(np.float32)
    V2 = (-2.0 * V).astype(np.float32)
    Wt2 = (-2.0 * W.T).astype(np.float32)
    bias0 = (2.0 * (b.astype(np.float64) @ V.astype(np.float64))).astype(
        np.float32)                      # dir0: +2*(b@V)
    bias1 = (-2.0 * b).astype(np.float32)

    # fp8 scales (powers of two, bounded to e4m3 range 240)
    mx = max(np.abs(protos1).max(), np.abs(protos2).max())
    sx = _pow2_below(224.0 / mx)
    n1 = np.sqrt((protos1.astype(np.float64) ** 2).sum(1))
    n2b = np.sqrt(((protos2.astype(np.float64) - b) ** 2).sum(1))
    colV = np.sqrt((V.astype(np.float64) ** 2).sum(0)).max()
    colW = np.sqrt((W.T.astype(np.float64) ** 2).sum(0)).max()
    B0 = 2.0 * n2b.max() * colV
    B1 = 2.0 * (n1.max() * colW + np.abs(b).max())
    sy0 = _pow2_below(224.0 / B0)
    sy1 = _pow2_below(224.0 / B1)

    # d-major class-sliced tables: (C, NPC, D) -> (C, D, NPC) -> (C,2,128,NPC)
    def dmajor(p):
        pc = p.reshape(NPC, C, D).transpose(1, 2, 0)      # (C, D, NPC)
        return np.ascontiguousarray(pc).reshape(C, 2, 128, NPC)

    p1t = dmajor(protos1)
    p2t = dmajor(protos2)
    q1t = (p1t * np.float32(sx)).astype(ml_dtypes.float8_e4m3)
    q2t = (p2t * np.float32(sx)).astype(ml_dtypes.float8_e4m3)

    # host xs: mean_i |x_i|^2 per class from the quantized tables
    def xsm_of(q):
        f = q.astype(np.float32).astype(np.float64) / sx
        return (f ** 2).sum(axis=(1, 2)).mean(axis=1)     # (C,)

    xsm = np.stack([xsm_of(q1t), xsm_of(q2t)]).astype(np.float64)  # (2, C)

    mats = np.stack([
        np.stack([V2[0:128, :], V2[128:256, :]]),
        np.stack([Wt2[0:128, :], Wt2[128:256, :]]),
    ]).astype(np.float32)                                 # [2, 2, 128, 256]
    ssq0 = math.sqrt(sx * sy0) / 2.0
    ssq1 = math.sqrt(sx * sy1) / 2.0
    biases = np.stack([
        np.concatenate([(bias0 * sy0).reshape(2, 128).T,
                        np.full((128, 1), sy0, np.float32),
                        (bias0 * ssq0).reshape(2, 128).T,
                        np.full((128, 1), ssq0, np.float32)], axis=1),
        np.concatenate([(bias1 * sy1).reshape(2, 128).T,
                        np.full((128, 1), sy1, np.float32),
                        (bias1 * ssq1).reshape(2, 128).T,
                        np.full((128, 1), ssq1, np.float32)], axis=1),
    ]).astype(np.float32)                                 # [2, 128, 6]
    onesc = np.ones((128, 3), dtype=np.float32)
    idb = np.eye(128, dtype=np.float32).astype(ml_dtypes.bfloat16)
    finsc = np.concatenate([
        np.full(CPC, 1.0 / (NPC * sx * sy0), np.float64),
        np.full(CPC, 1.0 / (NPC * sx * sy1), np.float64),
    ]).astype(np.float32).reshape(1, 2 * CPC)

    in_maps = []
    for core in range(N_CORES):
        sl = slice(core * CPC, (core + 1) * CPC)
        in_maps.append({
            "p1t": np.ascontiguousarray(p1t[sl]),
            "p2t": np.ascontiguousarray(p2t[sl]),
            "q1t": np.ascontiguousarray(q1t[sl]),
            "q2t": np.ascontiguousarray(q2t[sl]),
            "mats": mats,
            "biases": biases,
            "onesc": onesc,
            "idb": idb,
            "finsc": finsc,
        })

    nc = _get_nc()
    res = run_bass_kernel_spmd(nc, in_maps, core_ids=list(range(N_CORES)))
    _CACHE["last_result"] = res

    out = np.zeros((2, C), dtype=np.float64)
    for core in range(N_CORES):
        row = res.results[core]["out"].reshape(2, CPC).astype(np.float64)
        sl = slice(core * CPC, (core + 1) * CPC)
        out[0, sl] = row[0] + xsm[0, sl]
        out[1, sl] = row[1] + xsm[1, sl]
    return out.astype(np.float32)
